# revision 13
# baseline (speedup 1.0000x reference)
"""Trainium2 Bass kernel for a 16-head causal MultiHeadAttention block.

Problem (hardcoded): B=4, S=2048, D=1024, H=16, DK=64, fp32 I/O.
    out = softmax(mask(Q' K'^T / sqrt(DK))) V' @ Wo.T + bo
with Q' = Q@Wq.T+bq etc.

Sharding: 8 cores = (batch b = core//2, head-half = core%2).  Each core
computes its batch's q/k/v projections for its 8 heads, causal attention,
and a partial output projection over its 512 attn dims.  The host sums the
two partial outputs per batch (the Wo contraction distributes over heads).

Per-core kernel layout (v4, tuned against HW microbenches: all matmul
dtypes stream ~1 col/cycle at 2.4 GHz; fp8 DoubleRow contracts 256
rows/instr at 1 col/cycle; DR LDWEIGHTS loads 2x columns so DR only pays
off on wide streams; fp8/bf16 non-DR LDWEIGHTS get FWL 4x):
  - bf16 data path everywhere except: q/k projections for sequence tiles
    st >= 1 run fp8e4m3 DoubleRow (256-row contraction - half the
    instructions).  Host emulation: the fp8 score noise only hurts
    low-key-count (early) rows, so st=0 stays bf16; max rel err 4.3e-3
    vs the 2e-2 budget.  fp8 V or output projections fail the budget
    (their noise hits the output directly) so those stay bf16.
  - Host pre-transposes activations (X^T [D,S]) / weights and pre-scales
    Wq, Wk by 2^8 (fp8 resolution); exp scale 2^-19 folds that and
    1/sqrt(DK).
  - q/k land transposed [head_dim, seq] in bf16; scores are computed
    transposed (scoresT[k, q] = kT.T @ qT) so exp output feeds PV
    directly as lhsT.
  - Score matmuls for the head-pair's two heads (partitions 0:64 / 64:128)
    are issued adjacently: they map to different PE row-groups and run
    concurrently (~2x).  Each (jp, s_) score tile [128, 2, 512] holds both
    heads; one exp instruction (N=1024) covers both, bias -1.5 (softmax
    shift-invariance; keeps exp small, scores bounded |s| < 6.5).
  - A slice of the full-tile exps runs on the DVE instead (Schraudolph
    int-bit-trick exp, ~3% rel err, inside the bf16-path budget) to
    offload the Activation engine.
  - PV runs bf16 with per-j v tiles [128, HPC, DK+1]; the ones column
    accumulates the softmax denominator for free.
  - Diagonal 128x128 blocks: exp for both heads in one instruction,
    triangular zero mask via gpsimd affine_select.
  - Causal structure at tile granularity: upper-triangle k-tiles skipped.
  - Normalization fused: one scalar_tensor_tensor per (hp, sub) with a
    broadcast reciprocal; bf16 PE transposes (1 cyc/row) into a single
    psum tile; one attnT copy per (qt, hp).
"""

import math
import contextlib

import numpy as np
import ml_dtypes
import concourse.bacc as bacc
import concourse.tile as tile
from concourse import mybir
from concourse import bass_utils
from concourse.masks import make_identity
from concourse.bass import broadcast_tensor_aps

B, S, D, H = 4, 2048, 1024, 16
DK = D // H            # 64
NCORES = 8
HPC = H // 2           # 8 heads per core
DHC = HPC * DK         # 512 attn dims per core
KD = D // 128          # 8 contraction chunks (bf16 path)
KD2 = D // 256         # 4 DoubleRow contraction chunks (fp8 path)
NPAIR = HPC // 2       # 4 head pairs per core
EXPBIAS = -1.5
SCALE_QK = float(2.0 ** -19)   # 1/(2^8 * 2^8 * sqrt(DK))
_A_SCH = 12102203.16158        # 2^23/ln(2)
SCHRA_A = _A_SCH * SCALE_QK
SCHRA_B = float(127 * 2 ** 23 - 366393 + EXPBIAS * _A_SCH)
QK_FP8_ST0 = 1                 # first seq tile using the fp8 qk path
SCHRA_MOD, SCHRA_LIM = 14, 3   # DVE-exp share of full-tile exps
INTERLEAVE_PROJ = True         # issue st>=1 projections inside attention

F32 = mybir.dt.float32
F32R = mybir.dt.float32r
BF16 = mybir.dt.bfloat16
FP8 = mybir.dt.float8e4
I32 = mybir.dt.int32
NPF8 = ml_dtypes.float8_e4m3
NPBF16 = ml_dtypes.bfloat16
DR = mybir.MatmulPerfMode.DoubleRow


def build_nc(seq=S, causal=True, repeat=1):
    nc = bacc.Bacc("TRN2", target_bir_lowering=False, debug=False)

    NKC = seq // 128   # 128-wide k/seq tiles
    NQT = seq // 512   # 512-wide q tiles
    NJP = NKC // 2     # j-block pairs

    xq = nc.dram_tensor("xqT", [D, seq], BF16, kind="ExternalInput").ap()
    xk = nc.dram_tensor("xkT", [D, seq], BF16, kind="ExternalInput").ap()
    xv = nc.dram_tensor("xvT", [D, seq], BF16, kind="ExternalInput").ap()
    xq8 = nc.dram_tensor("xqT8", [D, seq], FP8, kind="ExternalInput").ap()
    xk8 = nc.dram_tensor("xkT8", [D, seq], FP8, kind="ExternalInput").ap()
    wq = nc.dram_tensor("wqT", [D, DHC], BF16, kind="ExternalInput").ap()
    wk = nc.dram_tensor("wkT", [D, DHC], BF16, kind="ExternalInput").ap()
    wq8 = nc.dram_tensor("wqT8", [D, DHC], FP8, kind="ExternalInput").ap()
    wk8 = nc.dram_tensor("wkT8", [D, DHC], FP8, kind="ExternalInput").ap()
    wv = nc.dram_tensor("wvT", [D, DHC], BF16, kind="ExternalInput").ap()
    wo = nc.dram_tensor("woT", [DHC, D], BF16, kind="ExternalInput").ap()
    out = nc.dram_tensor("out", [seq, D], BF16, kind="ExternalOutput").ap()

    EXP = mybir.ActivationFunctionType.Exp

    with tile.TileContext(nc) as tc, contextlib.ExitStack() as ctx:
        ep = ctx.enter_context

        consts = ep(tc.tile_pool(name="consts", bufs=1))
        wpool = ep(tc.tile_pool(name="wpool", bufs=2))
        w8pool = ep(tc.tile_pool(name="w8pool", bufs=2))
        wopool = ep(tc.tile_pool(name="wopool", bufs=1))
        xpool = ep(tc.tile_pool(name="xpool", bufs=6))
        qtp = ep(tc.tile_pool(name="qtp", bufs=NPAIR))
        ktp = ep(tc.tile_pool(name="ktp", bufs=NPAIR))
        vbp = ep(tc.tile_pool(name="vbp", bufs=NKC))
        atp = ep(tc.tile_pool(name="atp", bufs=2 * NPAIR + 4))
        pt2p = ep(tc.tile_pool(name="pt2p", bufs=8))
        ptdp = ep(tc.tile_pool(name="ptdp", bufs=6))
        stgp = ep(tc.tile_pool(name="stgp", bufs=6))
        outp = ep(tc.tile_pool(name="outp", bufs=6))
        rcp = ep(tc.tile_pool(name="rcp", bufs=8))
        sstp = ep(tc.tile_pool(name="sstp", bufs=3))
        psA = ep(tc.tile_pool(name="psA", bufs=2, space="PSUM"))
        psB = ep(tc.tile_pool(name="psB", bufs=2, space="PSUM"))
        psD = ep(tc.tile_pool(name="psD", bufs=1, space="PSUM"))

        ident = consts.tile([128, 128], BF16)
        make_identity(nc, ident)
        ebias = consts.tile([128, 1], F32, tag="ebias", name="ebias")
        nc.gpsimd.memset(ebias, EXPBIAS)

        for rep_i in range(repeat):
            def load_w16(wdram, dma_eng):
                wsb = wpool.tile([128, KD, DHC], BF16, tag="w", name="wsb")
                dma_eng.dma_start(
                    out=wsb, in_=wdram.rearrange("(c p) m -> p c m", p=128))
                return wsb

            def load_w8(wdram, dma_eng):
                wsb = w8pool.tile([128, KD2, 2, DHC], FP8, tag="w8",
                                  name="wsb8")
                dma_eng.dma_start(
                    out=wsb,
                    in_=wdram.rearrange("(c two p) m -> p c two m",
                                        p=128, two=2))
                return wsb

            wq_sb = load_w16(wq, nc.scalar)
            wk_sb = load_w16(wk, nc.gpsimd)
            wq8_sb = load_w8(wq8, nc.scalar)
            wk8_sb = load_w8(wk8, nc.gpsimd)
            wo_sb = wopool.tile([128, DHC // 128, D], BF16)
            nc.sync.dma_start(out=wo_sb, in_=wo.rearrange("(c p) n -> p c n", p=128))
            # (out DMAs below keep nc.sync mostly to themselves)

            qT = [qtp.tile([128, seq], BF16, tag="qT", name=f"qT{i}") for i in range(NPAIR)]
            kT = [ktp.tile([128, seq], BF16, tag="kT", name=f"kT{i}") for i in range(NPAIR)]

            # ---- q / k projections: bf16 for st < QK_FP8_ST0, fp8
            # DoubleRow beyond.  psum[dpair, s] = sum_i W[i, dpair] X[i, s]
            def qk_proj(dst, xdram, x8dram, wsb, w8sb, dma_eng):
                for st in range(seq // 512):
                    f8path = st >= QK_FP8_ST0
                    if f8path:
                        xt = xpool.tile([128, KD2, 2, 512], FP8, tag="x",
                                        name="xt8")
                        dma_eng.dma_start(
                            out=xt,
                            in_=x8dram[:, st * 512:(st + 1) * 512].rearrange(
                                "(c two p) s -> p c two s", p=128, two=2))
                    else:
                        xt = xpool.tile([128, KD, 512], BF16, tag="x",
                                        name="xt")
                        dma_eng.dma_start(
                            out=xt,
                            in_=xdram[:, st * 512:(st + 1) * 512].rearrange(
                                "(c p) s -> p c s", p=128))
                    for pair in range(NPAIR):
                        ps = psA.tile([128, 512], F32, tag="psA", name="psp")
                        if f8path:
                            for kc in range(KD2):
                                nc.tensor.matmul(
                                    ps,
                                    lhsT=w8sb[:, kc, :,
                                              pair * 128:(pair + 1) * 128],
                                    rhs=xt[:, kc, :, :],
                                    start=(kc == 0), stop=(kc == KD2 - 1),
                                    perf_mode=DR,
                                )
                        else:
                            for kc in range(KD):
                                nc.tensor.matmul(
                                    ps,
                                    lhsT=wsb[:, kc,
                                             pair * 128:(pair + 1) * 128],
                                    rhs=xt[:, kc, :],
                                    start=(kc == 0), stop=(kc == KD - 1),
                                )
                        nc.vector.tensor_copy(
                            out=dst[pair][:, st * 512:(st + 1) * 512], in_=ps)

            def qk_units(dst, xdram, x8dram, wsb, w8sb, dma_eng, st):
                f8path = st >= QK_FP8_ST0
                if f8path:
                    xt = xpool.tile([128, KD2, 2, 512], FP8, tag="x",
                                    name="xt8")
                    dma_eng.dma_start(
                        out=xt,
                        in_=x8dram[:, st * 512:(st + 1) * 512].rearrange(
                            "(c two p) s -> p c two s", p=128, two=2))
                else:
                    xt = xpool.tile([128, KD, 512], BF16, tag="x", name="xt")
                    dma_eng.dma_start(
                        out=xt,
                        in_=xdram[:, st * 512:(st + 1) * 512].rearrange(
                            "(c p) s -> p c s", p=128))

                def unit(pair):
                    ps = psA.tile([128, 512], F32, tag="psA", name="psp")
                    if f8path:
                        for kc in range(KD2):
                            nc.tensor.matmul(
                                ps,
                                lhsT=w8sb[:, kc, :,
                                          pair * 128:(pair + 1) * 128],
                                rhs=xt[:, kc, :, :],
                                start=(kc == 0), stop=(kc == KD2 - 1),
                                perf_mode=DR,
                            )
                    else:
                        for kc in range(KD):
                            nc.tensor.matmul(
                                ps,
                                lhsT=wsb[:, kc, pair * 128:(pair + 1) * 128],
                                rhs=xt[:, kc, :],
                                start=(kc == 0), stop=(kc == KD - 1),
                            )
                    nc.vector.tensor_copy(
                        out=dst[pair][:, st * 512:(st + 1) * 512], in_=ps)
                return [lambda pair=pair: unit(pair) for pair in range(NPAIR)]

            # ---- v projection (natural layout, bf16) + ones column
            wv_sb = load_w16(wv, nc.gpsimd)
            vb = [None] * NKC

            def v_units(st):
                xt = xpool.tile([128, KD, 512], BF16, tag="x", name="xt")
                nc.gpsimd.dma_start(
                    out=xt,
                    in_=xv[:, st * 512:(st + 1) * 512].rearrange(
                        "(c p) s -> p c s", p=128))

                def unit(sq):
                    j = st * 4 + sq
                    ps = psA.tile([128, 512], F32, tag="psA", name="psp")
                    for kc in range(KD):
                        nc.tensor.matmul(
                            ps,
                            lhsT=xt[:, kc, sq * 128:(sq + 1) * 128],
                            rhs=wv_sb[:, kc, :],
                            start=(kc == 0), stop=(kc == KD - 1),
                        )
                    psh = ps.rearrange("p (h d) -> p h d", h=HPC)
                    vb[j] = vbp.tile([128, HPC, DK + 1], BF16, tag="vb",
                                     name="vbt")
                    nc.gpsimd.memset(vb[j][:, :, DK:DK + 1], 1.0)
                    nc.vector.tensor_copy(out=vb[j][:, :, 0:DK], in_=psh)
                return [lambda sq=sq: unit(sq) for sq in range(4)]

            # st=0 projections run eagerly (attention(qt=0) needs them);
            # later sequence tiles are interleaved into the attention loop
            # below so projection matmuls fill the PE while the Activation
            # engine works through the exp stream.
            pending = []
            for u in qk_units(qT, xq, xq8, wq_sb, wq8_sb, nc.scalar, 0):
                u()
            for u in qk_units(kT, xk, xk8, wk_sb, wk8_sb, nc.gpsimd, 0):
                u()
            for u in v_units(0):
                u()
            for st in range(1, seq // 512):
                pending.extend(qk_units(qT, xq, xq8, wq_sb, wq8_sb,
                                        nc.scalar, st))
                pending.extend(qk_units(kT, xk, xk8, wk_sb, wk8_sb,
                                        nc.gpsimd, st))
                pending.extend(v_units(st))
            # 36 pending units over 16 (qt, hp) iterations; the st=qt+1
            # slice is issued during attention(qt): 12 units per qt = 3/hp
            if not INTERLEAVE_PROJ:
                for u in pending:
                    u()
                pending = []

            if not causal:
                for u in pending:
                    u()
            # ---- attention + output projection, one 512-wide q tile at a time
            for qt in range(NQT):
                njp = 2 * qt if causal else NJP
                attnT = [atp.tile([128, 512], BF16, tag="attnT", name=f"attnT{i}") for i in range(NPAIR)]
                for hp in range(NPAIR):
                    if causal:
                        base = qt * 12
                        for u in pending[base + hp * 3:base + hp * 3 + 3]:
                            u()
                    stg = stgp.tile([128, 4, 128], BF16, tag="stg", name="stg")
                    pvs = [psB.tile([128, 4, DK + 1], F32, tag="psB", name="pvt")
                           for _ in range(2)]
                    first = True
                    # full 128-key j tiles: both heads' scores in one
                    # 2-bank psum tile -> one exp -> bf16 FWL PV matmuls
                    for jp in range(njp):
                        for s_ in range(2):
                            j = 2 * jp + s_
                            sc = psA.tile([128, 2, 512], F32, tag="psA",
                                          name="sct")
                            for sub in range(2):
                                row0 = sub * 64
                                nc.tensor.matmul(
                                    sc[:, sub, :],
                                    lhsT=kT[hp][row0:row0 + 64,
                                                j * 128:(j + 1) * 128],
                                    rhs=qT[hp][row0:row0 + 64,
                                               qt * 512:(qt + 1) * 512],
                                    start=True,
                                    stop=True,
                                )
                            pt2 = pt2p.tile([128, 2, 512], BF16, tag="pt2",
                                            name="pt2")
                            if (j * 3 + hp) % SCHRA_MOD < SCHRA_LIM:
                                # Schraudolph exp on the DVE: int bit-trick
                                #   i32 = A*u + B;  f32-bits(i32) ~ e^u
                                ss = sstp.tile([128, 2, 512], I32,
                                               tag="ss", name="ss")
                                nc.vector.tensor_scalar(
                                    out=ss, in0=sc,
                                    scalar1=SCHRA_A, scalar2=SCHRA_B,
                                    op0=mybir.AluOpType.mult,
                                    op1=mybir.AluOpType.add)
                                nc.vector.tensor_copy(
                                    out=pt2, in_=ss.bitcast(F32))
                            else:
                                nc.scalar.activation(pt2, sc, EXP, bias=ebias,
                                                     scale=SCALE_QK)
                            for sub in range(2):
                                h = hp * 2 + sub
                                for c in range(4):
                                    nc.tensor.matmul(
                                        pvs[sub][:, c, :],
                                        lhsT=pt2[:, sub,
                                                 c * 128:(c + 1) * 128],
                                        rhs=vb[j][:, h, :],
                                        start=first and c == 0,
                                        stop=(not causal and jp == njp - 1
                                              and s_ == 1 and c == 3),
                                    )
                            first = False
                    # diagonal blocks: both heads per exp, triangular
                    # mask on the 128x128 diagonal block
                    if causal:
                        for d in range(4):
                            j = 4 * qt + d
                            qoff = d * 128
                            w = 512 - qoff
                            sc2 = psD.tile([128, 2, 512], F32, tag="psD",
                                           name="scd")
                            for sub in range(2):
                                row0 = sub * 64
                                nc.tensor.matmul(
                                    sc2[:, sub, qoff:512],
                                    lhsT=kT[hp][row0:row0 + 64,
                                                j * 128:(j + 1) * 128],
                                    rhs=qT[hp][row0:row0 + 64,
                                               qt * 512 + qoff:(qt + 1) * 512],
                                    start=True,
                                    stop=True,
                                )
                            ptd = ptdp.tile([128, 2, 512], BF16, tag="ptd",
                                            name="ptd")
                            nc.scalar.activation(
                                ptd[:, :, 0:w], sc2[:, :, qoff:512], EXP,
                                bias=ebias, scale=SCALE_QK)
                            # diagonal 128x128 block: zero p where k > q
                            for sub in range(2):
                                nc.gpsimd.affine_select(
                                    out=ptd[:, sub, 0:128],
                                    in_=ptd[:, sub, 0:128],
                                    compare_op=mybir.AluOpType.is_ge,
                                    fill=0.0,
                                    base=0,
                                    channel_multiplier=-1,
                                    pattern=[[1, 128]],
                                )
                            for sub in range(2):
                                h = hp * 2 + sub
                                for c in range(d, 4):
                                    nc.tensor.matmul(
                                        pvs[sub][:, c, :],
                                        lhsT=ptd[:, sub, c * 128 - qoff:
                                                 c * 128 - qoff + 128],
                                        rhs=vb[j][:, h, :],
                                        start=first and d == 0 and c == d,
                                        stop=(d == 3 and c == 3),
                                    )
                            first = False
                    for sub in range(2):
                        row0 = sub * 64
                        rc4 = rcp.tile([128, 4, 1], F32, tag="rc", name="rc")
                        nc.vector.reciprocal(rc4, pvs[sub][:, :, DK:DK + 1])
                        pv_in = pvs[sub][:, :, 0:DK]
                        rc_b, pv_b = broadcast_tensor_aps(
                            rc4[:, :, 0:1], pv_in)
                        nc.vector.scalar_tensor_tensor(
                            out=stg[:, :, row0:row0 + 64],
                            in0=pv_b, scalar=1.0, in1=rc_b,
                            op0=mybir.AluOpType.mult,
                            op1=mybir.AluOpType.mult)
                    tpv = psA.tile([128, 4, 128], BF16, tag="psA", name="tp")
                    for c in range(4):
                        nc.tensor.transpose(tpv[:, c, :], stg[:, c, :], ident)
                    nc.vector.tensor_copy(
                        out=attnT[hp].rearrange("p (c n) -> p c n", c=4),
                        in_=tpv)

                for t in range(4):
                    row = (qt * 4 + t) * 128
                    ps2 = psD.tile([128, 2, 512], F32, tag="psD", name="pso")
                    for half in range(2):
                        for dc in range(NPAIR):
                            nc.tensor.matmul(
                                ps2[:, half, :],
                                lhsT=attnT[dc][:, t * 128:(t + 1) * 128],
                                rhs=wo_sb[:, dc, half * 512:(half + 1) * 512],
                                start=(dc == 0),
                                stop=(dc == NPAIR - 1),
                            )
                    og = outp.tile([128, D], BF16, tag="out")
                    nc.vector.tensor_copy(
                        out=og.rearrange("p (h n) -> p h n", h=2), in_=ps2)
                    nc.sync.dma_start(out=out[row:row + 128, :], in_=og)

    nc.compile()
    return nc


_NC_CACHE = {}


def _get_nc(seq, causal, repeat=1):
    key = (seq, causal, repeat)
    if key not in _NC_CACHE:
        _NC_CACHE[key] = build_nc(seq, causal, repeat)
    return _NC_CACHE[key]


def shard_inputs(Q, K, V, Wq, Wk, Wv, Wo, seq=S):
    xT = {}
    for b in range(B):
        xT[b] = (
            np.asarray(Q[b][:seq].T, dtype=NPBF16, order="C"),
            np.asarray(K[b][:seq].T, dtype=NPBF16, order="C"),
            np.asarray(V[b][:seq].T, dtype=NPBF16, order="C"),
            np.asarray(Q[b][:seq].T, dtype=NPF8, order="C"),
            np.asarray(K[b][:seq].T, dtype=NPF8, order="C"),
        )
    wT = {}
    for hh in range(2):
        ds0 = hh * DHC
        wq_s = (Wq[ds0:ds0 + DHC] * 256.0).T
        wk_s = (Wk[ds0:ds0 + DHC] * 256.0).T
        wT[hh] = (
            np.asarray(wq_s, dtype=NPBF16, order="C"),
            np.asarray(wk_s, dtype=NPBF16, order="C"),
            np.asarray(wq_s, dtype=NPF8, order="C"),
            np.asarray(wk_s, dtype=NPF8, order="C"),
            np.asarray(Wv[ds0:ds0 + DHC].T, dtype=NPBF16, order="C"),
            np.asarray(Wo[:, ds0:ds0 + DHC].T, dtype=NPBF16, order="C"),
        )
    in_maps = []
    for c in range(NCORES):
        b, hh = c // 2, c % 2
        in_maps.append({
            "xqT": xT[b][0], "xkT": xT[b][1], "xvT": xT[b][2],
            "xqT8": xT[b][3], "xkT8": xT[b][4],
            "wqT": wT[hh][0], "wkT": wT[hh][1],
            "wqT8": wT[hh][2], "wkT8": wT[hh][3],
            "wvT": wT[hh][4], "woT": wT[hh][5],
        })
    return in_maps


def _numpy_ref(Q, K, V, mask, Wq, bq, Wk, bk, Wv, bv, Wo, bo):
    """Safety-net host fallback for input patterns the device kernel
    doesn't handle (non-causal non-empty masks, nonzero q/k biases)."""
    b = Q.shape[0]
    q = (Q @ Wq.T + bq).reshape(b, -1, H, DK).transpose(0, 2, 1, 3)
    k = (K @ Wk.T + bk).reshape(b, -1, H, DK).transpose(0, 2, 1, 3)
    v = (V @ Wv.T + bv).reshape(b, -1, H, DK).transpose(0, 2, 1, 3)
    scores = np.einsum("bhqd,bhkd->bhqk", q, k) / math.sqrt(DK)
    scores = np.where(mask, np.float32(-1e9), scores)
    scores -= scores.max(axis=-1, keepdims=True)
    p = np.exp(scores)
    p /= p.sum(axis=-1, keepdims=True)
    o = np.einsum("bhqk,bhkd->bhqd", p, v)
    o = o.transpose(0, 2, 1, 3).reshape(b, -1, H * DK)
    return (o @ Wo.T + bo).astype(np.float32)


def _run(inputs, trace=False):
    Q = np.asarray(inputs["Q"], np.float32)
    K = np.asarray(inputs["K"], np.float32)
    V = np.asarray(inputs["V"], np.float32)
    mask = np.asarray(inputs["mask"], bool)
    Wq = np.asarray(inputs["Wq"], np.float32)
    bq = np.asarray(inputs["bq"], np.float32)
    Wk = np.asarray(inputs["Wk"], np.float32)
    bk = np.asarray(inputs["bk"], np.float32)
    Wv = np.asarray(inputs["Wv"], np.float32)
    bv = np.asarray(inputs["bv"], np.float32)
    Wo = np.asarray(inputs["Wo"], np.float32)
    bo = np.asarray(inputs["bo"], np.float32)

    seq = Q.shape[1]
    m2 = mask[:, 0]
    triu = np.triu(np.ones((seq, seq), bool), 1)
    if all(np.array_equal(m2[i], triu) for i in range(m2.shape[0])):
        causal = True
    elif not mask.any():
        causal = False
    else:
        return _numpy_ref(Q, K, V, mask, Wq, bq, Wk, bk, Wv, bv, Wo, bo), None
    if bq.any() or bk.any():
        return _numpy_ref(Q, K, V, mask, Wq, bq, Wk, bk, Wv, bv, Wo, bo), None

    nc = _get_nc(seq, causal)
    in_maps = shard_inputs(Q, K, V, Wq, Wk, Wv, Wo, seq)
    res = bass_utils.run_bass_kernel_spmd(
        nc, in_maps, core_ids=list(range(NCORES)), trace=trace
    )
    outs = [np.asarray(r["out"], np.float32) for r in res.results]
    out = np.empty((B, seq, D), np.float32)
    for b in range(B):
        out[b] = outs[2 * b] + outs[2 * b + 1]
    # v-bias distributes through softmax (weights sum to 1); o-bias is direct
    out += bo + bv @ Wo.T
    return out, res


def kernel(**inputs):
    out, _ = _run(inputs)
    return out


def make_timed_runner(nc, in_maps):
    """Build a jitted shard_map callable over 8 cores with device-resident,
    non-donated inputs, for steady-state kernel timing (no NTFF hook is
    available under this axon client, so wall-clock the sharded executable)."""
    import jax
    from jax.experimental.shard_map import shard_map
    from jax.sharding import Mesh, NamedSharding, PartitionSpec
    from concourse import bass2jax
    from concourse import mybir as mb

    bass2jax.install_neuronx_cc_hook()

    partition_name = (
        nc.partition_id_tensor.name if nc.partition_id_tensor else None
    )
    in_names, out_names, out_avals, zero_outs = [], [], [], []
    for alloc in nc.m.functions[0].allocations:
        if not isinstance(alloc, mb.MemoryLocationSet):
            continue
        name = alloc.memorylocations[0].name
        if alloc.kind == "ExternalInput":
            if name != partition_name:
                in_names.append(name)
        elif alloc.kind == "ExternalOutput":
            out_names.append(name)
            out_avals.append(
                jax.core.ShapedArray(tuple(alloc.tensor_shape), mb.dt.np(alloc.dtype))
            )
            zero_outs.append(
                np.zeros(tuple(alloc.tensor_shape), mb.dt.np(alloc.dtype))
            )
    n_params = len(in_names)
    all_names = in_names + out_names
    if partition_name is not None:
        all_names = all_names + [partition_name]

    def _body(*args):
        operands = list(args)
        if partition_name is not None:
            operands.append(bass2jax.partition_id_tensor())
        outs = bass2jax._bass_exec_p.bind(
            *operands,
            out_avals=tuple(out_avals),
            in_names=tuple(all_names),
            out_names=tuple(out_names),
            lowering_input_output_aliases=(),
            sim_require_finite=True,
            sim_require_nnan=True,
            nc=nc,
        )
        return tuple(outs)

    n = len(in_maps)
    devices = jax.devices()[:n]
    mesh = Mesh(np.asarray(devices), ("core",))
    spec = PartitionSpec("core")
    sharded = jax.jit(
        shard_map(
            _body,
            mesh=mesh,
            in_specs=(spec,) * (n_params + len(out_names)),
            out_specs=(spec,) * len(out_names),
            check_rep=False,
        ),
        keep_unused=True,
    )
    sh = NamedSharding(mesh, spec)
    args = [
        jax.device_put(
            np.concatenate([np.asarray(m[nm]) for m in in_maps], axis=0), sh
        )
        for nm in in_names
    ] + [
        jax.device_put(
            np.zeros((n * z.shape[0], *z.shape[1:]), z.dtype), sh
        )
        for z in zero_outs
    ]
    return sharded, args


# revision 15
# speedup vs baseline: 1.2950x; 1.2950x over previous
"""Trainium2 Bass kernel for a 16-head causal MultiHeadAttention block.

Problem (hardcoded): B=4, S=2048, D=1024, H=16, DK=64, fp32 I/O.
    out = softmax(mask(Q' K'^T / sqrt(DK))) V' @ Wo.T + bo
with Q' = Q@Wq.T+bq etc.

Sharding: 8 cores = (batch b = core//2, head-half = core%2).  Each core
computes its batch's q/k/v projections for its 8 heads, causal attention,
and a partial output projection over its 512 attn dims.  The host sums the
two partial outputs per batch (the Wo contraction distributes over heads).

Per-core kernel layout (v4, tuned against HW microbenches: all matmul
dtypes stream ~1 col/cycle at 2.4 GHz; fp8 DoubleRow contracts 256
rows/instr at 1 col/cycle; DR LDWEIGHTS loads 2x columns so DR only pays
off on wide streams; fp8/bf16 non-DR LDWEIGHTS get FWL 4x):
  - bf16 data path everywhere except: q/k projections for sequence tiles
    st >= 1 run fp8e4m3 DoubleRow (256-row contraction - half the
    instructions).  Host emulation: the fp8 score noise only hurts
    low-key-count (early) rows, so st=0 stays bf16; max rel err 4.3e-3
    vs the 2e-2 budget.  fp8 V or output projections fail the budget
    (their noise hits the output directly) so those stay bf16.
  - Host pre-transposes activations (X^T [D,S]) / weights and pre-scales
    Wq, Wk by 2^8 (fp8 resolution); exp scale 2^-19 folds that and
    1/sqrt(DK).
  - q/k land transposed [head_dim, seq] in bf16; scores are computed
    transposed (scoresT[k, q] = kT.T @ qT) so exp output feeds PV
    directly as lhsT.
  - Score matmuls for the head-pair's two heads (partitions 0:64 / 64:128)
    are issued adjacently: they map to different PE row-groups and run
    concurrently (~2x).  Each (jp, s_) score tile [128, 2, 512] holds both
    heads; one exp instruction (N=1024) covers both, bias -1.5 (softmax
    shift-invariance; keeps exp small, scores bounded |s| < 6.5).
  - A slice of the full-tile exps runs on the DVE instead (Schraudolph
    int-bit-trick exp, ~3% rel err, inside the bf16-path budget) to
    offload the Activation engine.
  - PV runs bf16 with per-j v tiles [128, HPC, DK+1]; the ones column
    accumulates the softmax denominator for free.
  - Diagonal 128x128 blocks: exp for both heads in one instruction,
    triangular zero mask via gpsimd affine_select.
  - Causal structure at tile granularity: upper-triangle k-tiles skipped.
  - Normalization fused: one scalar_tensor_tensor per (hp, sub) with a
    broadcast reciprocal; bf16 PE transposes (1 cyc/row) into a single
    psum tile; one attnT copy per (qt, hp).
"""

import math
import contextlib

import numpy as np
import ml_dtypes
import concourse.bacc as bacc
import concourse.tile as tile
from concourse import mybir
from concourse import bass_utils
from concourse.masks import make_identity
from concourse.bass import broadcast_tensor_aps

B, S, D, H = 4, 2048, 1024, 16
DK = D // H            # 64
NCORES = 8
HPC = H // 2           # 8 heads per core
DHC = HPC * DK         # 512 attn dims per core
KD = D // 128          # 8 contraction chunks (bf16 path)
KD2 = D // 256         # 4 DoubleRow contraction chunks (fp8 path)
NPAIR = HPC // 2       # 4 head pairs per core
EXPBIAS = -1.5
SCALE_QK = float(2.0 ** -19)   # 1/(2^8 * 2^8 * sqrt(DK))
_A_SCH = 12102203.16158        # 2^23/ln(2)
SCHRA_A = _A_SCH * SCALE_QK
SCHRA_B = float(127 * 2 ** 23 - 366393 + EXPBIAS * _A_SCH)
QK_FP8_ST0 = 1                 # first seq tile using the fp8 qk path
SCHRA_MOD, SCHRA_LIM = 14, 5   # DVE-exp share of full-tile exps
INTERLEAVE_PROJ = True         # issue st>=1 projections inside attention

F32 = mybir.dt.float32
F32R = mybir.dt.float32r
BF16 = mybir.dt.bfloat16
FP8 = mybir.dt.float8e4
I32 = mybir.dt.int32
NPF8 = ml_dtypes.float8_e4m3
NPBF16 = ml_dtypes.bfloat16
DR = mybir.MatmulPerfMode.DoubleRow


def build_nc(seq=S, causal=True, repeat=1):
    nc = bacc.Bacc("TRN2", target_bir_lowering=False, debug=False)

    NKC = seq // 128   # 128-wide k/seq tiles
    NQT = seq // 512   # 512-wide q tiles
    NJP = NKC // 2     # j-block pairs

    xq = nc.dram_tensor("xqT", [D, seq], BF16, kind="ExternalInput").ap()
    xk = nc.dram_tensor("xkT", [D, seq], BF16, kind="ExternalInput").ap()
    xv = nc.dram_tensor("xvT", [D, seq], BF16, kind="ExternalInput").ap()
    xq8 = nc.dram_tensor("xqT8", [D, seq], FP8, kind="ExternalInput").ap()
    xk8 = nc.dram_tensor("xkT8", [D, seq], FP8, kind="ExternalInput").ap()
    wq = nc.dram_tensor("wqT", [D, DHC], BF16, kind="ExternalInput").ap()
    wk = nc.dram_tensor("wkT", [D, DHC], BF16, kind="ExternalInput").ap()
    wq8 = nc.dram_tensor("wqT8", [D, DHC], FP8, kind="ExternalInput").ap()
    wk8 = nc.dram_tensor("wkT8", [D, DHC], FP8, kind="ExternalInput").ap()
    wv = nc.dram_tensor("wvT", [D, DHC], BF16, kind="ExternalInput").ap()
    wo = nc.dram_tensor("woT", [DHC, D], BF16, kind="ExternalInput").ap()
    out = nc.dram_tensor("out", [seq, D], BF16, kind="ExternalOutput").ap()

    EXP = mybir.ActivationFunctionType.Exp

    with tile.TileContext(nc) as tc, contextlib.ExitStack() as ctx:
        ep = ctx.enter_context

        consts = ep(tc.tile_pool(name="consts", bufs=1))
        wpool = ep(tc.tile_pool(name="wpool", bufs=2))
        w8pool = ep(tc.tile_pool(name="w8pool", bufs=2))
        wopool = ep(tc.tile_pool(name="wopool", bufs=1))
        xpool = ep(tc.tile_pool(name="xpool", bufs=4))
        qtp = ep(tc.tile_pool(name="qtp", bufs=NPAIR))
        ktp = ep(tc.tile_pool(name="ktp", bufs=NPAIR))
        vbp = ep(tc.tile_pool(name="vbp", bufs=NKC))
        atp = ep(tc.tile_pool(name="atp", bufs=2 * NPAIR))
        pt2p = ep(tc.tile_pool(name="pt2p", bufs=6))
        ptdp = ep(tc.tile_pool(name="ptdp", bufs=4))
        stgp = ep(tc.tile_pool(name="stgp", bufs=4))
        outp = ep(tc.tile_pool(name="outp", bufs=4))
        rcp = ep(tc.tile_pool(name="rcp", bufs=4))
        sstp = ep(tc.tile_pool(name="sstp", bufs=3))
        psA = ep(tc.tile_pool(name="psA", bufs=2, space="PSUM"))
        psB = ep(tc.tile_pool(name="psB", bufs=2, space="PSUM"))
        psD = ep(tc.tile_pool(name="psD", bufs=1, space="PSUM"))

        ident = consts.tile([128, 128], BF16)
        make_identity(nc, ident)
        ebias = consts.tile([128, 1], F32, tag="ebias", name="ebias")
        nc.gpsimd.memset(ebias, EXPBIAS)

        for rep_i in range(repeat):
            def load_w16(wdram, dma_eng):
                wsb = wpool.tile([128, KD, DHC], BF16, tag="w", name="wsb")
                dma_eng.dma_start(
                    out=wsb, in_=wdram.rearrange("(c p) m -> p c m", p=128))
                return wsb

            def load_w8(wdram, dma_eng):
                wsb = w8pool.tile([128, KD2, 2, DHC], FP8, tag="w8",
                                  name="wsb8")
                dma_eng.dma_start(
                    out=wsb,
                    in_=wdram.rearrange("(c two p) m -> p c two m",
                                        p=128, two=2))
                return wsb

            wq_sb = load_w16(wq, nc.scalar)
            wk_sb = load_w16(wk, nc.gpsimd)
            wq8_sb = load_w8(wq8, nc.scalar)
            wk8_sb = load_w8(wk8, nc.gpsimd)
            wo_sb = wopool.tile([128, DHC // 128, D], BF16)
            nc.sync.dma_start(out=wo_sb, in_=wo.rearrange("(c p) n -> p c n", p=128))
            # (out DMAs below keep nc.sync mostly to themselves)

            qT = [qtp.tile([128, seq], BF16, tag="qT", name=f"qT{i}") for i in range(NPAIR)]
            kT = [ktp.tile([128, seq], BF16, tag="kT", name=f"kT{i}") for i in range(NPAIR)]

            # ---- q / k projections: bf16 for st < QK_FP8_ST0, fp8
            # DoubleRow beyond.  psum[dpair, s] = sum_i W[i, dpair] X[i, s]
            def qk_proj(dst, xdram, x8dram, wsb, w8sb, dma_eng):
                for st in range(seq // 512):
                    f8path = st >= QK_FP8_ST0
                    if f8path:
                        xt = xpool.tile([128, KD2, 2, 512], FP8, tag="x",
                                        name="xt8")
                        dma_eng.dma_start(
                            out=xt,
                            in_=x8dram[:, st * 512:(st + 1) * 512].rearrange(
                                "(c two p) s -> p c two s", p=128, two=2))
                    else:
                        xt = xpool.tile([128, KD, 512], BF16, tag="x",
                                        name="xt")
                        dma_eng.dma_start(
                            out=xt,
                            in_=xdram[:, st * 512:(st + 1) * 512].rearrange(
                                "(c p) s -> p c s", p=128))
                    for pair in range(NPAIR):
                        ps = psA.tile([128, 512], F32, tag="psA", name="psp")
                        if f8path:
                            for kc in range(KD2):
                                nc.tensor.matmul(
                                    ps,
                                    lhsT=w8sb[:, kc, :,
                                              pair * 128:(pair + 1) * 128],
                                    rhs=xt[:, kc, :, :],
                                    start=(kc == 0), stop=(kc == KD2 - 1),
                                    perf_mode=DR,
                                )
                        else:
                            for kc in range(KD):
                                nc.tensor.matmul(
                                    ps,
                                    lhsT=wsb[:, kc,
                                             pair * 128:(pair + 1) * 128],
                                    rhs=xt[:, kc, :],
                                    start=(kc == 0), stop=(kc == KD - 1),
                                )
                        nc.vector.tensor_copy(
                            out=dst[pair][:, st * 512:(st + 1) * 512], in_=ps)

            def qk_units(dst, xdram, x8dram, wsb, w8sb, dma_eng, st):
                f8path = st >= QK_FP8_ST0
                if f8path:
                    xt = xpool.tile([128, KD2, 2, 512], FP8, tag="x",
                                    name="xt8")
                    dma_eng.dma_start(
                        out=xt,
                        in_=x8dram[:, st * 512:(st + 1) * 512].rearrange(
                            "(c two p) s -> p c two s", p=128, two=2))
                else:
                    xt = xpool.tile([128, KD, 512], BF16, tag="x", name="xt")
                    dma_eng.dma_start(
                        out=xt,
                        in_=xdram[:, st * 512:(st + 1) * 512].rearrange(
                            "(c p) s -> p c s", p=128))

                def unit(pair):
                    ps = psA.tile([128, 512], F32, tag="psA", name="psp")
                    if f8path:
                        for kc in range(KD2):
                            nc.tensor.matmul(
                                ps,
                                lhsT=w8sb[:, kc, :,
                                          pair * 128:(pair + 1) * 128],
                                rhs=xt[:, kc, :, :],
                                start=(kc == 0), stop=(kc == KD2 - 1),
                                perf_mode=DR,
                            )
                    else:
                        for kc in range(KD):
                            nc.tensor.matmul(
                                ps,
                                lhsT=wsb[:, kc, pair * 128:(pair + 1) * 128],
                                rhs=xt[:, kc, :],
                                start=(kc == 0), stop=(kc == KD - 1),
                            )
                    nc.vector.tensor_copy(
                        out=dst[pair][:, st * 512:(st + 1) * 512], in_=ps)
                return [lambda pair=pair: unit(pair) for pair in range(NPAIR)]

            # ---- v projection (natural layout, bf16) + ones column
            wv_sb = load_w16(wv, nc.gpsimd)
            vb = [None] * NKC

            def v_units(st):
                xt = xpool.tile([128, KD, 512], BF16, tag="x", name="xt")
                nc.gpsimd.dma_start(
                    out=xt,
                    in_=xv[:, st * 512:(st + 1) * 512].rearrange(
                        "(c p) s -> p c s", p=128))

                def unit(sq):
                    j = st * 4 + sq
                    ps = psA.tile([128, 512], F32, tag="psA", name="psp")
                    for kc in range(KD):
                        nc.tensor.matmul(
                            ps,
                            lhsT=xt[:, kc, sq * 128:(sq + 1) * 128],
                            rhs=wv_sb[:, kc, :],
                            start=(kc == 0), stop=(kc == KD - 1),
                        )
                    psh = ps.rearrange("p (h d) -> p h d", h=HPC)
                    vb[j] = vbp.tile([128, HPC, DK + 1], BF16, tag="vb",
                                     name="vbt")
                    nc.gpsimd.memset(vb[j][:, :, DK:DK + 1], 1.0)
                    nc.vector.tensor_copy(out=vb[j][:, :, 0:DK], in_=psh)
                return [lambda sq=sq: unit(sq) for sq in range(4)]

            # st=0 projections run eagerly (attention(qt=0) needs them);
            # later sequence tiles are interleaved into the attention loop
            # below so projection matmuls fill the PE while the Activation
            # engine works through the exp stream.
            pending = []
            for u in qk_units(qT, xq, xq8, wq_sb, wq8_sb, nc.scalar, 0):
                u()
            for u in qk_units(kT, xk, xk8, wk_sb, wk8_sb, nc.gpsimd, 0):
                u()
            for u in v_units(0):
                u()
            for st in range(1, seq // 512):
                pending.extend(qk_units(qT, xq, xq8, wq_sb, wq8_sb,
                                        nc.scalar, st))
                pending.extend(qk_units(kT, xk, xk8, wk_sb, wk8_sb,
                                        nc.gpsimd, st))
                pending.extend(v_units(st))
            # 36 pending units over 16 (qt, hp) iterations; the st=qt+1
            # slice is issued during attention(qt): 12 units per qt = 3/hp
            if not INTERLEAVE_PROJ:
                for u in pending:
                    u()
                pending = []

            if not causal:
                for u in pending:
                    u()
            # ---- attention + output projection, one 512-wide q tile at a time
            for qt in range(NQT):
                njp = 2 * qt if causal else NJP
                attnT = [atp.tile([128, 512], BF16, tag="attnT", name=f"attnT{i}") for i in range(NPAIR)]
                for hp in range(NPAIR):
                    if causal:
                        base = qt * 12
                        for u in pending[base + hp * 3:base + hp * 3 + 3]:
                            u()
                    stg = stgp.tile([128, 4, 128], BF16, tag="stg", name="stg")
                    pvs = [psB.tile([128, 4, DK + 1], F32, tag="psB", name="pvt")
                           for _ in range(2)]
                    first = True
                    # full 128-key j tiles: both heads' scores in one
                    # 2-bank psum tile -> one exp -> bf16 FWL PV matmuls
                    for jp in range(njp):
                        for s_ in range(2):
                            j = 2 * jp + s_
                            sc = psA.tile([128, 2, 512], F32, tag="psA",
                                          name="sct")
                            for sub in range(2):
                                row0 = sub * 64
                                nc.tensor.matmul(
                                    sc[:, sub, :],
                                    lhsT=kT[hp][row0:row0 + 64,
                                                j * 128:(j + 1) * 128],
                                    rhs=qT[hp][row0:row0 + 64,
                                               qt * 512:(qt + 1) * 512],
                                    start=True,
                                    stop=True,
                                )
                            pt2 = pt2p.tile([128, 2, 512], BF16, tag="pt2",
                                            name="pt2")
                            if (j * 3 + hp) % SCHRA_MOD < SCHRA_LIM:
                                # Schraudolph exp on the DVE: int bit-trick
                                #   i32 = A*u + B;  f32-bits(i32) ~ e^u
                                ss = sstp.tile([128, 2, 512], I32,
                                               tag="ss", name="ss")
                                nc.vector.tensor_scalar(
                                    out=ss, in0=sc,
                                    scalar1=SCHRA_A, scalar2=SCHRA_B,
                                    op0=mybir.AluOpType.mult,
                                    op1=mybir.AluOpType.add)
                                nc.vector.tensor_copy(
                                    out=pt2, in_=ss.bitcast(F32))
                            else:
                                nc.scalar.activation(pt2, sc, EXP, bias=ebias,
                                                     scale=SCALE_QK)
                            for sub in range(2):
                                h = hp * 2 + sub
                                for c in range(4):
                                    nc.tensor.matmul(
                                        pvs[sub][:, c, :],
                                        lhsT=pt2[:, sub,
                                                 c * 128:(c + 1) * 128],
                                        rhs=vb[j][:, h, :],
                                        start=first and c == 0,
                                        stop=(not causal and jp == njp - 1
                                              and s_ == 1 and c == 3),
                                    )
                            first = False
                    # diagonal blocks: both heads per exp, triangular
                    # mask on the 128x128 diagonal block
                    if causal:
                        for d in range(4):
                            j = 4 * qt + d
                            qoff = d * 128
                            w = 512 - qoff
                            sc2 = psD.tile([128, 2, 512], F32, tag="psD",
                                           name="scd")
                            for sub in range(2):
                                row0 = sub * 64
                                nc.tensor.matmul(
                                    sc2[:, sub, qoff:512],
                                    lhsT=kT[hp][row0:row0 + 64,
                                                j * 128:(j + 1) * 128],
                                    rhs=qT[hp][row0:row0 + 64,
                                               qt * 512 + qoff:(qt + 1) * 512],
                                    start=True,
                                    stop=True,
                                )
                            ptd = ptdp.tile([128, 2, 512], BF16, tag="ptd",
                                            name="ptd")
                            nc.scalar.activation(
                                ptd[:, :, 0:w], sc2[:, :, qoff:512], EXP,
                                bias=ebias, scale=SCALE_QK)
                            # diagonal 128x128 block: zero p where k > q
                            for sub in range(2):
                                nc.gpsimd.affine_select(
                                    out=ptd[:, sub, 0:128],
                                    in_=ptd[:, sub, 0:128],
                                    compare_op=mybir.AluOpType.is_ge,
                                    fill=0.0,
                                    base=0,
                                    channel_multiplier=-1,
                                    pattern=[[1, 128]],
                                )
                            for sub in range(2):
                                h = hp * 2 + sub
                                for c in range(d, 4):
                                    nc.tensor.matmul(
                                        pvs[sub][:, c, :],
                                        lhsT=ptd[:, sub, c * 128 - qoff:
                                                 c * 128 - qoff + 128],
                                        rhs=vb[j][:, h, :],
                                        start=first and d == 0 and c == d,
                                        stop=(d == 3 and c == 3),
                                    )
                            first = False
                    for sub in range(2):
                        row0 = sub * 64
                        rc4 = rcp.tile([128, 4, 1], F32, tag="rc", name="rc")
                        nc.vector.reciprocal(rc4, pvs[sub][:, :, DK:DK + 1])
                        pv_in = pvs[sub][:, :, 0:DK]
                        rc_b, pv_b = broadcast_tensor_aps(
                            rc4[:, :, 0:1], pv_in)
                        nc.vector.scalar_tensor_tensor(
                            out=stg[:, :, row0:row0 + 64],
                            in0=pv_b, scalar=1.0, in1=rc_b,
                            op0=mybir.AluOpType.mult,
                            op1=mybir.AluOpType.mult)
                    tpv = psA.tile([128, 4, 128], BF16, tag="psA", name="tp")
                    for c in range(4):
                        nc.tensor.transpose(tpv[:, c, :], stg[:, c, :], ident)
                    nc.vector.tensor_copy(
                        out=attnT[hp].rearrange("p (c n) -> p c n", c=4),
                        in_=tpv)

                for t in range(4):
                    row = (qt * 4 + t) * 128
                    ps2 = psD.tile([128, 2, 512], F32, tag="psD", name="pso")
                    for half in range(2):
                        for dc in range(NPAIR):
                            nc.tensor.matmul(
                                ps2[:, half, :],
                                lhsT=attnT[dc][:, t * 128:(t + 1) * 128],
                                rhs=wo_sb[:, dc, half * 512:(half + 1) * 512],
                                start=(dc == 0),
                                stop=(dc == NPAIR - 1),
                            )
                    og = outp.tile([128, D], BF16, tag="out")
                    nc.vector.tensor_copy(
                        out=og.rearrange("p (h n) -> p h n", h=2), in_=ps2)
                    nc.sync.dma_start(out=out[row:row + 128, :], in_=og)

    nc.compile()
    return nc


_NC_CACHE = {}


def _get_nc(seq, causal, repeat=1):
    key = (seq, causal, repeat)
    if key not in _NC_CACHE:
        _NC_CACHE[key] = build_nc(seq, causal, repeat)
    return _NC_CACHE[key]


def shard_inputs(Q, K, V, Wq, Wk, Wv, Wo, seq=S):
    xT = {}
    for b in range(B):
        xT[b] = (
            np.asarray(Q[b][:seq].T, dtype=NPBF16, order="C"),
            np.asarray(K[b][:seq].T, dtype=NPBF16, order="C"),
            np.asarray(V[b][:seq].T, dtype=NPBF16, order="C"),
            np.asarray(Q[b][:seq].T, dtype=NPF8, order="C"),
            np.asarray(K[b][:seq].T, dtype=NPF8, order="C"),
        )
    wT = {}
    for hh in range(2):
        ds0 = hh * DHC
        wq_s = (Wq[ds0:ds0 + DHC] * 256.0).T
        wk_s = (Wk[ds0:ds0 + DHC] * 256.0).T
        wT[hh] = (
            np.asarray(wq_s, dtype=NPBF16, order="C"),
            np.asarray(wk_s, dtype=NPBF16, order="C"),
            np.asarray(wq_s, dtype=NPF8, order="C"),
            np.asarray(wk_s, dtype=NPF8, order="C"),
            np.asarray(Wv[ds0:ds0 + DHC].T, dtype=NPBF16, order="C"),
            np.asarray(Wo[:, ds0:ds0 + DHC].T, dtype=NPBF16, order="C"),
        )
    in_maps = []
    for c in range(NCORES):
        b, hh = c // 2, c % 2
        in_maps.append({
            "xqT": xT[b][0], "xkT": xT[b][1], "xvT": xT[b][2],
            "xqT8": xT[b][3], "xkT8": xT[b][4],
            "wqT": wT[hh][0], "wkT": wT[hh][1],
            "wqT8": wT[hh][2], "wkT8": wT[hh][3],
            "wvT": wT[hh][4], "woT": wT[hh][5],
        })
    return in_maps


def _numpy_ref(Q, K, V, mask, Wq, bq, Wk, bk, Wv, bv, Wo, bo):
    """Safety-net host fallback for input patterns the device kernel
    doesn't handle (non-causal non-empty masks, nonzero q/k biases)."""
    b = Q.shape[0]
    q = (Q @ Wq.T + bq).reshape(b, -1, H, DK).transpose(0, 2, 1, 3)
    k = (K @ Wk.T + bk).reshape(b, -1, H, DK).transpose(0, 2, 1, 3)
    v = (V @ Wv.T + bv).reshape(b, -1, H, DK).transpose(0, 2, 1, 3)
    scores = np.einsum("bhqd,bhkd->bhqk", q, k) / math.sqrt(DK)
    scores = np.where(mask, np.float32(-1e9), scores)
    scores -= scores.max(axis=-1, keepdims=True)
    p = np.exp(scores)
    p /= p.sum(axis=-1, keepdims=True)
    o = np.einsum("bhqk,bhkd->bhqd", p, v)
    o = o.transpose(0, 2, 1, 3).reshape(b, -1, H * DK)
    return (o @ Wo.T + bo).astype(np.float32)


def _run(inputs, trace=False):
    Q = np.asarray(inputs["Q"], np.float32)
    K = np.asarray(inputs["K"], np.float32)
    V = np.asarray(inputs["V"], np.float32)
    mask = np.asarray(inputs["mask"], bool)
    Wq = np.asarray(inputs["Wq"], np.float32)
    bq = np.asarray(inputs["bq"], np.float32)
    Wk = np.asarray(inputs["Wk"], np.float32)
    bk = np.asarray(inputs["bk"], np.float32)
    Wv = np.asarray(inputs["Wv"], np.float32)
    bv = np.asarray(inputs["bv"], np.float32)
    Wo = np.asarray(inputs["Wo"], np.float32)
    bo = np.asarray(inputs["bo"], np.float32)

    seq = Q.shape[1]
    m2 = mask[:, 0]
    triu = np.triu(np.ones((seq, seq), bool), 1)
    if all(np.array_equal(m2[i], triu) for i in range(m2.shape[0])):
        causal = True
    elif not mask.any():
        causal = False
    else:
        return _numpy_ref(Q, K, V, mask, Wq, bq, Wk, bk, Wv, bv, Wo, bo), None
    if bq.any() or bk.any():
        return _numpy_ref(Q, K, V, mask, Wq, bq, Wk, bk, Wv, bv, Wo, bo), None

    nc = _get_nc(seq, causal)
    in_maps = shard_inputs(Q, K, V, Wq, Wk, Wv, Wo, seq)
    res = bass_utils.run_bass_kernel_spmd(
        nc, in_maps, core_ids=list(range(NCORES)), trace=trace
    )
    outs = [np.asarray(r["out"], np.float32) for r in res.results]
    out = np.empty((B, seq, D), np.float32)
    for b in range(B):
        out[b] = outs[2 * b] + outs[2 * b + 1]
    # v-bias distributes through softmax (weights sum to 1); o-bias is direct
    out += bo + bv @ Wo.T
    return out, res


def kernel(**inputs):
    out, _ = _run(inputs)
    return out


def make_timed_runner(nc, in_maps):
    """Build a jitted shard_map callable over 8 cores with device-resident,
    non-donated inputs, for steady-state kernel timing (no NTFF hook is
    available under this axon client, so wall-clock the sharded executable)."""
    import jax
    from jax.experimental.shard_map import shard_map
    from jax.sharding import Mesh, NamedSharding, PartitionSpec
    from concourse import bass2jax
    from concourse import mybir as mb

    bass2jax.install_neuronx_cc_hook()

    partition_name = (
        nc.partition_id_tensor.name if nc.partition_id_tensor else None
    )
    in_names, out_names, out_avals, zero_outs = [], [], [], []
    for alloc in nc.m.functions[0].allocations:
        if not isinstance(alloc, mb.MemoryLocationSet):
            continue
        name = alloc.memorylocations[0].name
        if alloc.kind == "ExternalInput":
            if name != partition_name:
                in_names.append(name)
        elif alloc.kind == "ExternalOutput":
            out_names.append(name)
            out_avals.append(
                jax.core.ShapedArray(tuple(alloc.tensor_shape), mb.dt.np(alloc.dtype))
            )
            zero_outs.append(
                np.zeros(tuple(alloc.tensor_shape), mb.dt.np(alloc.dtype))
            )
    n_params = len(in_names)
    all_names = in_names + out_names
    if partition_name is not None:
        all_names = all_names + [partition_name]

    def _body(*args):
        operands = list(args)
        if partition_name is not None:
            operands.append(bass2jax.partition_id_tensor())
        outs = bass2jax._bass_exec_p.bind(
            *operands,
            out_avals=tuple(out_avals),
            in_names=tuple(all_names),
            out_names=tuple(out_names),
            lowering_input_output_aliases=(),
            sim_require_finite=True,
            sim_require_nnan=True,
            nc=nc,
        )
        return tuple(outs)

    n = len(in_maps)
    devices = jax.devices()[:n]
    mesh = Mesh(np.asarray(devices), ("core",))
    spec = PartitionSpec("core")
    sharded = jax.jit(
        shard_map(
            _body,
            mesh=mesh,
            in_specs=(spec,) * (n_params + len(out_names)),
            out_specs=(spec,) * len(out_names),
            check_rep=False,
        ),
        keep_unused=True,
    )
    sh = NamedSharding(mesh, spec)
    args = [
        jax.device_put(
            np.concatenate([np.asarray(m[nm]) for m in in_maps], axis=0), sh
        )
        for nm in in_names
    ] + [
        jax.device_put(
            np.zeros((n * z.shape[0], *z.shape[1:]), z.dtype), sh
        )
        for z in zero_outs
    ]
    return sharded, args


# revision 16
# speedup vs baseline: 1.6167x; 1.2484x over previous
"""Trainium2 Bass kernel for a 16-head causal MultiHeadAttention block.

Problem (hardcoded): B=4, S=2048, D=1024, H=16, DK=64, fp32 I/O.
    out = softmax(mask(Q' K'^T / sqrt(DK))) V' @ Wo.T + bo
with Q' = Q@Wq.T+bq etc.

Sharding: 8 cores = (batch b = core//2, head-half = core%2).  Each core
computes its batch's q/k/v projections for its 8 heads, causal attention,
and a partial output projection over its 512 attn dims.  The host sums the
two partial outputs per batch (the Wo contraction distributes over heads).

Per-core kernel layout (v4, tuned against HW microbenches: all matmul
dtypes stream ~1 col/cycle at 2.4 GHz; fp8 DoubleRow contracts 256
rows/instr at 1 col/cycle; DR LDWEIGHTS loads 2x columns so DR only pays
off on wide streams; fp8/bf16 non-DR LDWEIGHTS get FWL 4x):
  - bf16 data path everywhere except: q/k projections for sequence tiles
    st >= 1 run fp8e4m3 DoubleRow (256-row contraction - half the
    instructions).  Host emulation: the fp8 score noise only hurts
    low-key-count (early) rows, so st=0 stays bf16; max rel err 4.3e-3
    vs the 2e-2 budget.  fp8 V or output projections fail the budget
    (their noise hits the output directly) so those stay bf16.
  - Host pre-transposes activations (X^T [D,S]) / weights and pre-scales
    Wq, Wk by 2^8 (fp8 resolution); exp scale 2^-19 folds that and
    1/sqrt(DK).
  - q/k land transposed [head_dim, seq] in bf16; scores are computed
    transposed (scoresT[k, q] = kT.T @ qT) so exp output feeds PV
    directly as lhsT.
  - Score matmuls for the head-pair's two heads (partitions 0:64 / 64:128)
    are issued adjacently: they map to different PE row-groups and run
    concurrently (~2x).  Each (jp, s_) score tile [128, 2, 512] holds both
    heads; one exp instruction (N=1024) covers both, bias -1.5 (softmax
    shift-invariance; keeps exp small, scores bounded |s| < 6.5).
  - A slice of the full-tile exps runs on the DVE instead (Schraudolph
    int-bit-trick exp, ~3% rel err, inside the bf16-path budget) to
    offload the Activation engine.
  - PV runs bf16 with per-j v tiles [128, HPC, DK+1]; the ones column
    accumulates the softmax denominator for free.
  - Diagonal 128x128 blocks: exp for both heads in one instruction,
    triangular zero mask via gpsimd affine_select.
  - Causal structure at tile granularity: upper-triangle k-tiles skipped.
  - Normalization fused: one scalar_tensor_tensor per (hp, sub) with a
    broadcast reciprocal; bf16 PE transposes (1 cyc/row) into a single
    psum tile; one attnT copy per (qt, hp).
"""

import math
import contextlib

import numpy as np
import ml_dtypes
import concourse.bacc as bacc
import concourse.tile as tile
from concourse import mybir
from concourse import bass_utils
from concourse.masks import make_identity
from concourse.bass import broadcast_tensor_aps

B, S, D, H = 4, 2048, 1024, 16
DK = D // H            # 64
NCORES = 8
HPC = H // 2           # 8 heads per core
DHC = HPC * DK         # 512 attn dims per core
KD = D // 128          # 8 contraction chunks (bf16 path)
KD2 = D // 256         # 4 DoubleRow contraction chunks (fp8 path)
NPAIR = HPC // 2       # 4 head pairs per core
EXPBIAS = -1.5
SCALE_QK = float(2.0 ** -19)   # 1/(2^8 * 2^8 * sqrt(DK))
_A_SCH = 12102203.16158        # 2^23/ln(2)
SCHRA_A = _A_SCH * SCALE_QK
SCHRA_B = float(127 * 2 ** 23 - 366393 + EXPBIAS * _A_SCH)
QK_FP8_ST0 = 1                 # first seq tile using the fp8 qk path
SCHRA_MOD, SCHRA_LIM = 14, 5   # DVE-exp share of full-tile exps
INTERLEAVE_PROJ = True         # issue st>=1 projections inside attention

F32 = mybir.dt.float32
F32R = mybir.dt.float32r
BF16 = mybir.dt.bfloat16
FP8 = mybir.dt.float8e4
I32 = mybir.dt.int32
NPF8 = ml_dtypes.float8_e4m3
NPBF16 = ml_dtypes.bfloat16
DR = mybir.MatmulPerfMode.DoubleRow


def build_nc(seq=S, causal=True, repeat=1):
    nc = bacc.Bacc("TRN2", target_bir_lowering=False, debug=False)

    NKC = seq // 128   # 128-wide k/seq tiles
    NQT = seq // 512   # 512-wide q tiles
    NJP = NKC // 2     # j-block pairs

    xq = nc.dram_tensor("xqT", [D, seq], BF16, kind="ExternalInput").ap()
    xk = nc.dram_tensor("xkT", [D, seq], BF16, kind="ExternalInput").ap()
    xv = nc.dram_tensor("xvT", [D, seq], BF16, kind="ExternalInput").ap()
    xq8 = nc.dram_tensor("xqT8", [D, seq], FP8, kind="ExternalInput").ap()
    xk8 = nc.dram_tensor("xkT8", [D, seq], FP8, kind="ExternalInput").ap()
    wq = nc.dram_tensor("wqT", [D, DHC], BF16, kind="ExternalInput").ap()
    wk = nc.dram_tensor("wkT", [D, DHC], BF16, kind="ExternalInput").ap()
    wq8 = nc.dram_tensor("wqT8", [D, DHC], FP8, kind="ExternalInput").ap()
    wk8 = nc.dram_tensor("wkT8", [D, DHC], FP8, kind="ExternalInput").ap()
    wv = nc.dram_tensor("wvT", [D, DHC], BF16, kind="ExternalInput").ap()
    wo = nc.dram_tensor("woT", [DHC, D], BF16, kind="ExternalInput").ap()
    out = nc.dram_tensor("out", [seq, D], BF16, kind="ExternalOutput").ap()

    EXP = mybir.ActivationFunctionType.Exp

    with tile.TileContext(nc) as tc, contextlib.ExitStack() as ctx:
        ep = ctx.enter_context

        consts = ep(tc.tile_pool(name="consts", bufs=1))
        wpool = ep(tc.tile_pool(name="wpool", bufs=2))
        w8pool = ep(tc.tile_pool(name="w8pool", bufs=2))
        wopool = ep(tc.tile_pool(name="wopool", bufs=1))
        xpool = ep(tc.tile_pool(name="xpool", bufs=4))
        qtp = ep(tc.tile_pool(name="qtp", bufs=NPAIR))
        ktp = ep(tc.tile_pool(name="ktp", bufs=NPAIR))
        vbp = ep(tc.tile_pool(name="vbp", bufs=NKC))
        atp = ep(tc.tile_pool(name="atp", bufs=2 * NPAIR))
        pt2p = ep(tc.tile_pool(name="pt2p", bufs=6))
        ptdp = ep(tc.tile_pool(name="ptdp", bufs=4))
        stgp = ep(tc.tile_pool(name="stgp", bufs=4))
        outp = ep(tc.tile_pool(name="outp", bufs=4))
        rcp = ep(tc.tile_pool(name="rcp", bufs=4))
        sstp = ep(tc.tile_pool(name="sstp", bufs=3))
        psA = ep(tc.tile_pool(name="psA", bufs=2, space="PSUM"))
        psB = ep(tc.tile_pool(name="psB", bufs=2, space="PSUM"))
        psD = ep(tc.tile_pool(name="psD", bufs=1, space="PSUM"))

        ident = consts.tile([128, 128], BF16)
        make_identity(nc, ident)
        ebias = consts.tile([128, 1], F32, tag="ebias", name="ebias")
        nc.gpsimd.memset(ebias, EXPBIAS)

        for rep_i in range(repeat):
            def load_w16(wdram, dma_eng):
                wsb = wpool.tile([128, KD, DHC], BF16, tag="w", name="wsb")
                dma_eng.dma_start(
                    out=wsb, in_=wdram.rearrange("(c p) m -> p c m", p=128))
                return wsb

            def load_w8(wdram, dma_eng):
                wsb = w8pool.tile([128, KD2, 2, DHC], FP8, tag="w8",
                                  name="wsb8")
                dma_eng.dma_start(
                    out=wsb,
                    in_=wdram.rearrange("(c two p) m -> p c two m",
                                        p=128, two=2))
                return wsb

            wq_sb = load_w16(wq, nc.scalar)
            wk_sb = load_w16(wk, nc.gpsimd)
            wq8_sb = load_w8(wq8, nc.scalar)
            wk8_sb = load_w8(wk8, nc.gpsimd)
            wo_sb = wopool.tile([128, DHC // 128, D], BF16)
            nc.sync.dma_start(out=wo_sb, in_=wo.rearrange("(c p) n -> p c n", p=128))
            # (out DMAs below keep nc.sync mostly to themselves)

            qT = [qtp.tile([128, seq], BF16, tag="qT", name=f"qT{i}") for i in range(NPAIR)]
            kT = [ktp.tile([128, seq], BF16, tag="kT", name=f"kT{i}") for i in range(NPAIR)]

            # ---- q / k projections: bf16 for st < QK_FP8_ST0, fp8
            # DoubleRow beyond.  psum[dpair, s] = sum_i W[i, dpair] X[i, s]
            def qk_proj(dst, xdram, x8dram, wsb, w8sb, dma_eng):
                for st in range(seq // 512):
                    f8path = st >= QK_FP8_ST0
                    if f8path:
                        xt = xpool.tile([128, KD2, 2, 512], FP8, tag="x",
                                        name="xt8")
                        dma_eng.dma_start(
                            out=xt,
                            in_=x8dram[:, st * 512:(st + 1) * 512].rearrange(
                                "(c two p) s -> p c two s", p=128, two=2))
                    else:
                        xt = xpool.tile([128, KD, 512], BF16, tag="x",
                                        name="xt")
                        dma_eng.dma_start(
                            out=xt,
                            in_=xdram[:, st * 512:(st + 1) * 512].rearrange(
                                "(c p) s -> p c s", p=128))
                    for pair in range(NPAIR):
                        ps = psA.tile([128, 512], F32, tag="psA", name="psp")
                        if f8path:
                            for kc in range(KD2):
                                nc.tensor.matmul(
                                    ps,
                                    lhsT=w8sb[:, kc, :,
                                              pair * 128:(pair + 1) * 128],
                                    rhs=xt[:, kc, :, :],
                                    start=(kc == 0), stop=(kc == KD2 - 1),
                                    perf_mode=DR,
                                )
                        else:
                            for kc in range(KD):
                                nc.tensor.matmul(
                                    ps,
                                    lhsT=wsb[:, kc,
                                             pair * 128:(pair + 1) * 128],
                                    rhs=xt[:, kc, :],
                                    start=(kc == 0), stop=(kc == KD - 1),
                                )
                        nc.vector.tensor_copy(
                            out=dst[pair][:, st * 512:(st + 1) * 512], in_=ps)

            def qk_units(dst, xdram, x8dram, wsb, w8sb, dma_eng, st):
                f8path = st >= QK_FP8_ST0
                if f8path:
                    xt = xpool.tile([128, KD2, 2, 512], FP8, tag="x",
                                    name="xt8")
                    dma_eng.dma_start(
                        out=xt,
                        in_=x8dram[:, st * 512:(st + 1) * 512].rearrange(
                            "(c two p) s -> p c two s", p=128, two=2))
                else:
                    xt = xpool.tile([128, KD, 512], BF16, tag="x", name="xt")
                    dma_eng.dma_start(
                        out=xt,
                        in_=xdram[:, st * 512:(st + 1) * 512].rearrange(
                            "(c p) s -> p c s", p=128))

                def unit(pair):
                    ps = psA.tile([128, 512], F32, tag="psA", name="psp")
                    if f8path:
                        for kc in range(KD2):
                            nc.tensor.matmul(
                                ps,
                                lhsT=w8sb[:, kc, :,
                                          pair * 128:(pair + 1) * 128],
                                rhs=xt[:, kc, :, :],
                                start=(kc == 0), stop=(kc == KD2 - 1),
                                perf_mode=DR,
                            )
                    else:
                        for kc in range(KD):
                            nc.tensor.matmul(
                                ps,
                                lhsT=wsb[:, kc, pair * 128:(pair + 1) * 128],
                                rhs=xt[:, kc, :],
                                start=(kc == 0), stop=(kc == KD - 1),
                            )
                    nc.vector.tensor_copy(
                        out=dst[pair][:, st * 512:(st + 1) * 512], in_=ps)
                return [lambda pair=pair: unit(pair) for pair in range(NPAIR)]

            # ---- v projection (natural layout, bf16) + ones column
            wv_sb = load_w16(wv, nc.gpsimd)
            vb = [None] * NKC

            def v_units(st):
                xt = xpool.tile([128, KD, 512], BF16, tag="x", name="xt")
                nc.gpsimd.dma_start(
                    out=xt,
                    in_=xv[:, st * 512:(st + 1) * 512].rearrange(
                        "(c p) s -> p c s", p=128))

                def unit(sq):
                    j = st * 4 + sq
                    ps = psA.tile([128, 512], F32, tag="psA", name="psp")
                    for kc in range(KD):
                        nc.tensor.matmul(
                            ps,
                            lhsT=xt[:, kc, sq * 128:(sq + 1) * 128],
                            rhs=wv_sb[:, kc, :],
                            start=(kc == 0), stop=(kc == KD - 1),
                        )
                    psh = ps.rearrange("p (h d) -> p h d", h=HPC)
                    vb[j] = vbp.tile([128, HPC, DK + 1], BF16, tag="vb",
                                     name="vbt")
                    nc.gpsimd.memset(vb[j][:, :, DK:DK + 1], 1.0)
                    nc.vector.tensor_copy(out=vb[j][:, :, 0:DK], in_=psh)
                return [lambda sq=sq: unit(sq) for sq in range(4)]

            # st=0 projections run eagerly (attention(qt=0) needs them);
            # later sequence tiles are interleaved into the attention loop
            # below so projection matmuls fill the PE while the Activation
            # engine works through the exp stream.
            pending = []
            for u in qk_units(qT, xq, xq8, wq_sb, wq8_sb, nc.scalar, 0):
                u()
            for u in qk_units(kT, xk, xk8, wk_sb, wk8_sb, nc.gpsimd, 0):
                u()
            for u in v_units(0):
                u()
            for st in range(1, seq // 512):
                pending.extend(qk_units(qT, xq, xq8, wq_sb, wq8_sb,
                                        nc.scalar, st))
                pending.extend(qk_units(kT, xk, xk8, wk_sb, wk8_sb,
                                        nc.gpsimd, st))
                pending.extend(v_units(st))
            # 36 pending units over 16 (qt, hp) iterations; the st=qt+1
            # slice is issued during attention(qt): 12 units per qt = 3/hp
            if not INTERLEAVE_PROJ:
                for u in pending:
                    u()
                pending = []

            if not causal:
                for u in pending:
                    u()
            # ---- attention + output projection, one 512-wide q tile at a time
            for qt in range(NQT):
                njp = 2 * qt if causal else NJP
                attnT = [atp.tile([128, 512], BF16, tag="attnT", name=f"attnT{i}") for i in range(NPAIR)]
                for hp in range(NPAIR):
                    if causal:
                        base = qt * 12
                        for u in pending[base + hp * 3:base + hp * 3 + 3]:
                            u()
                    stg = stgp.tile([128, 4, 128], BF16, tag="stg", name="stg")
                    pvs = [psB.tile([128, 4, DK + 1], F32, tag="psB", name="pvt")
                           for _ in range(2)]
                    first = True
                    # full 128-key j tiles: both heads' scores in one
                    # 2-bank psum tile -> one exp -> bf16 FWL PV matmuls
                    for jp in range(njp):
                        for s_ in range(2):
                            j = 2 * jp + s_
                            sc = psA.tile([128, 2, 512], F32, tag="psA",
                                          name="sct")
                            for sub in range(2):
                                row0 = sub * 64
                                nc.tensor.matmul(
                                    sc[:, sub, :],
                                    lhsT=kT[hp][row0:row0 + 64,
                                                j * 128:(j + 1) * 128],
                                    rhs=qT[hp][row0:row0 + 64,
                                               qt * 512:(qt + 1) * 512],
                                    start=True,
                                    stop=True,
                                )
                            pt2 = pt2p.tile([128, 2, 512], BF16, tag="pt2",
                                            name="pt2")
                            if (j * 3 + hp) % SCHRA_MOD < SCHRA_LIM:
                                # Schraudolph exp on the DVE: int bit-trick
                                #   i32 = A*u + B;  f32-bits(i32) ~ e^u
                                ss = sstp.tile([128, 2, 512], I32,
                                               tag="ss", name="ss")
                                nc.vector.tensor_scalar(
                                    out=ss, in0=sc,
                                    scalar1=SCHRA_A, scalar2=SCHRA_B,
                                    op0=mybir.AluOpType.mult,
                                    op1=mybir.AluOpType.add)
                                nc.vector.tensor_copy(
                                    out=pt2, in_=ss.bitcast(F32))
                            else:
                                nc.scalar.activation(pt2, sc, EXP, bias=ebias,
                                                     scale=SCALE_QK)
                            for sub in range(2):
                                h = hp * 2 + sub
                                for c in range(4):
                                    nc.tensor.matmul(
                                        pvs[sub][:, c, :],
                                        lhsT=pt2[:, sub,
                                                 c * 128:(c + 1) * 128],
                                        rhs=vb[j][:, h, :],
                                        start=first and c == 0,
                                        stop=(not causal and jp == njp - 1
                                              and s_ == 1 and c == 3),
                                    )
                            first = False
                    # diagonal blocks: both heads per exp, triangular
                    # mask on the 128x128 diagonal block
                    if causal:
                        for d in range(4):
                            j = 4 * qt + d
                            qoff = d * 128
                            w = 512 - qoff
                            sc2 = psD.tile([128, 2, 512], F32, tag="psD",
                                           name="scd")
                            for sub in range(2):
                                row0 = sub * 64
                                nc.tensor.matmul(
                                    sc2[:, sub, qoff:512],
                                    lhsT=kT[hp][row0:row0 + 64,
                                                j * 128:(j + 1) * 128],
                                    rhs=qT[hp][row0:row0 + 64,
                                               qt * 512 + qoff:(qt + 1) * 512],
                                    start=True,
                                    stop=True,
                                )
                            ptd = ptdp.tile([128, 2, 512], BF16, tag="ptd",
                                            name="ptd")
                            nc.scalar.activation(
                                ptd[:, :, 0:w], sc2[:, :, qoff:512], EXP,
                                bias=ebias, scale=SCALE_QK)
                            # diagonal 128x128 block: zero p where k > q
                            for sub in range(2):
                                nc.gpsimd.affine_select(
                                    out=ptd[:, sub, 0:128],
                                    in_=ptd[:, sub, 0:128],
                                    compare_op=mybir.AluOpType.is_ge,
                                    fill=0.0,
                                    base=0,
                                    channel_multiplier=-1,
                                    pattern=[[1, 128]],
                                )
                            for sub in range(2):
                                h = hp * 2 + sub
                                for c in range(d, 4):
                                    nc.tensor.matmul(
                                        pvs[sub][:, c, :],
                                        lhsT=ptd[:, sub, c * 128 - qoff:
                                                 c * 128 - qoff + 128],
                                        rhs=vb[j][:, h, :],
                                        start=first and d == 0 and c == d,
                                        stop=(d == 3 and c == 3),
                                    )
                            first = False
                    for sub in range(2):
                        row0 = sub * 64
                        rc4 = rcp.tile([128, 4, 1], F32, tag="rc", name="rc")
                        nc.vector.reciprocal(rc4, pvs[sub][:, :, DK:DK + 1])
                        pv_in = pvs[sub][:, :, 0:DK]
                        rc_b, pv_b = broadcast_tensor_aps(
                            rc4[:, :, 0:1], pv_in)
                        nc.vector.scalar_tensor_tensor(
                            out=stg[:, :, row0:row0 + 64],
                            in0=pv_b, scalar=1.0, in1=rc_b,
                            op0=mybir.AluOpType.mult,
                            op1=mybir.AluOpType.mult)
                    tpv = psB.tile([128, 4, 128], BF16, tag="psB", name="tp")
                    for c in range(4):
                        nc.tensor.transpose(tpv[:, c, :], stg[:, c, :], ident)
                    nc.vector.tensor_copy(
                        out=attnT[hp].rearrange("p (c n) -> p c n", c=4),
                        in_=tpv)

                for t in range(4):
                    row = (qt * 4 + t) * 128
                    ps2 = psD.tile([128, 2, 512], F32, tag="psD", name="pso")
                    for half in range(2):
                        for dc in range(NPAIR):
                            nc.tensor.matmul(
                                ps2[:, half, :],
                                lhsT=attnT[dc][:, t * 128:(t + 1) * 128],
                                rhs=wo_sb[:, dc, half * 512:(half + 1) * 512],
                                start=(dc == 0),
                                stop=(dc == NPAIR - 1),
                            )
                    og = outp.tile([128, D], BF16, tag="out")
                    nc.vector.tensor_copy(
                        out=og.rearrange("p (h n) -> p h n", h=2), in_=ps2)
                    nc.sync.dma_start(out=out[row:row + 128, :], in_=og)

    nc.compile()
    return nc


_NC_CACHE = {}


def _get_nc(seq, causal, repeat=1):
    key = (seq, causal, repeat)
    if key not in _NC_CACHE:
        _NC_CACHE[key] = build_nc(seq, causal, repeat)
    return _NC_CACHE[key]


def shard_inputs(Q, K, V, Wq, Wk, Wv, Wo, seq=S):
    xT = {}
    for b in range(B):
        xT[b] = (
            np.asarray(Q[b][:seq].T, dtype=NPBF16, order="C"),
            np.asarray(K[b][:seq].T, dtype=NPBF16, order="C"),
            np.asarray(V[b][:seq].T, dtype=NPBF16, order="C"),
            np.asarray(Q[b][:seq].T, dtype=NPF8, order="C"),
            np.asarray(K[b][:seq].T, dtype=NPF8, order="C"),
        )
    wT = {}
    for hh in range(2):
        ds0 = hh * DHC
        wq_s = (Wq[ds0:ds0 + DHC] * 256.0).T
        wk_s = (Wk[ds0:ds0 + DHC] * 256.0).T
        wT[hh] = (
            np.asarray(wq_s, dtype=NPBF16, order="C"),
            np.asarray(wk_s, dtype=NPBF16, order="C"),
            np.asarray(wq_s, dtype=NPF8, order="C"),
            np.asarray(wk_s, dtype=NPF8, order="C"),
            np.asarray(Wv[ds0:ds0 + DHC].T, dtype=NPBF16, order="C"),
            np.asarray(Wo[:, ds0:ds0 + DHC].T, dtype=NPBF16, order="C"),
        )
    in_maps = []
    for c in range(NCORES):
        b, hh = c // 2, c % 2
        in_maps.append({
            "xqT": xT[b][0], "xkT": xT[b][1], "xvT": xT[b][2],
            "xqT8": xT[b][3], "xkT8": xT[b][4],
            "wqT": wT[hh][0], "wkT": wT[hh][1],
            "wqT8": wT[hh][2], "wkT8": wT[hh][3],
            "wvT": wT[hh][4], "woT": wT[hh][5],
        })
    return in_maps


def _numpy_ref(Q, K, V, mask, Wq, bq, Wk, bk, Wv, bv, Wo, bo):
    """Safety-net host fallback for input patterns the device kernel
    doesn't handle (non-causal non-empty masks, nonzero q/k biases)."""
    b = Q.shape[0]
    q = (Q @ Wq.T + bq).reshape(b, -1, H, DK).transpose(0, 2, 1, 3)
    k = (K @ Wk.T + bk).reshape(b, -1, H, DK).transpose(0, 2, 1, 3)
    v = (V @ Wv.T + bv).reshape(b, -1, H, DK).transpose(0, 2, 1, 3)
    scores = np.einsum("bhqd,bhkd->bhqk", q, k) / math.sqrt(DK)
    scores = np.where(mask, np.float32(-1e9), scores)
    scores -= scores.max(axis=-1, keepdims=True)
    p = np.exp(scores)
    p /= p.sum(axis=-1, keepdims=True)
    o = np.einsum("bhqk,bhkd->bhqd", p, v)
    o = o.transpose(0, 2, 1, 3).reshape(b, -1, H * DK)
    return (o @ Wo.T + bo).astype(np.float32)


def _run(inputs, trace=False):
    Q = np.asarray(inputs["Q"], np.float32)
    K = np.asarray(inputs["K"], np.float32)
    V = np.asarray(inputs["V"], np.float32)
    mask = np.asarray(inputs["mask"], bool)
    Wq = np.asarray(inputs["Wq"], np.float32)
    bq = np.asarray(inputs["bq"], np.float32)
    Wk = np.asarray(inputs["Wk"], np.float32)
    bk = np.asarray(inputs["bk"], np.float32)
    Wv = np.asarray(inputs["Wv"], np.float32)
    bv = np.asarray(inputs["bv"], np.float32)
    Wo = np.asarray(inputs["Wo"], np.float32)
    bo = np.asarray(inputs["bo"], np.float32)

    seq = Q.shape[1]
    m2 = mask[:, 0]
    triu = np.triu(np.ones((seq, seq), bool), 1)
    if all(np.array_equal(m2[i], triu) for i in range(m2.shape[0])):
        causal = True
    elif not mask.any():
        causal = False
    else:
        return _numpy_ref(Q, K, V, mask, Wq, bq, Wk, bk, Wv, bv, Wo, bo), None
    if bq.any() or bk.any():
        return _numpy_ref(Q, K, V, mask, Wq, bq, Wk, bk, Wv, bv, Wo, bo), None

    nc = _get_nc(seq, causal)
    in_maps = shard_inputs(Q, K, V, Wq, Wk, Wv, Wo, seq)
    res = bass_utils.run_bass_kernel_spmd(
        nc, in_maps, core_ids=list(range(NCORES)), trace=trace
    )
    outs = [np.asarray(r["out"], np.float32) for r in res.results]
    out = np.empty((B, seq, D), np.float32)
    for b in range(B):
        out[b] = outs[2 * b] + outs[2 * b + 1]
    # v-bias distributes through softmax (weights sum to 1); o-bias is direct
    out += bo + bv @ Wo.T
    return out, res


def kernel(**inputs):
    out, _ = _run(inputs)
    return out


def make_timed_runner(nc, in_maps):
    """Build a jitted shard_map callable over 8 cores with device-resident,
    non-donated inputs, for steady-state kernel timing (no NTFF hook is
    available under this axon client, so wall-clock the sharded executable)."""
    import jax
    from jax.experimental.shard_map import shard_map
    from jax.sharding import Mesh, NamedSharding, PartitionSpec
    from concourse import bass2jax
    from concourse import mybir as mb

    bass2jax.install_neuronx_cc_hook()

    partition_name = (
        nc.partition_id_tensor.name if nc.partition_id_tensor else None
    )
    in_names, out_names, out_avals, zero_outs = [], [], [], []
    for alloc in nc.m.functions[0].allocations:
        if not isinstance(alloc, mb.MemoryLocationSet):
            continue
        name = alloc.memorylocations[0].name
        if alloc.kind == "ExternalInput":
            if name != partition_name:
                in_names.append(name)
        elif alloc.kind == "ExternalOutput":
            out_names.append(name)
            out_avals.append(
                jax.core.ShapedArray(tuple(alloc.tensor_shape), mb.dt.np(alloc.dtype))
            )
            zero_outs.append(
                np.zeros(tuple(alloc.tensor_shape), mb.dt.np(alloc.dtype))
            )
    n_params = len(in_names)
    all_names = in_names + out_names
    if partition_name is not None:
        all_names = all_names + [partition_name]

    def _body(*args):
        operands = list(args)
        if partition_name is not None:
            operands.append(bass2jax.partition_id_tensor())
        outs = bass2jax._bass_exec_p.bind(
            *operands,
            out_avals=tuple(out_avals),
            in_names=tuple(all_names),
            out_names=tuple(out_names),
            lowering_input_output_aliases=(),
            sim_require_finite=True,
            sim_require_nnan=True,
            nc=nc,
        )
        return tuple(outs)

    n = len(in_maps)
    devices = jax.devices()[:n]
    mesh = Mesh(np.asarray(devices), ("core",))
    spec = PartitionSpec("core")
    sharded = jax.jit(
        shard_map(
            _body,
            mesh=mesh,
            in_specs=(spec,) * (n_params + len(out_names)),
            out_specs=(spec,) * len(out_names),
            check_rep=False,
        ),
        keep_unused=True,
    )
    sh = NamedSharding(mesh, spec)
    args = [
        jax.device_put(
            np.concatenate([np.asarray(m[nm]) for m in in_maps], axis=0), sh
        )
        for nm in in_names
    ] + [
        jax.device_put(
            np.zeros((n * z.shape[0], *z.shape[1:]), z.dtype), sh
        )
        for z in zero_outs
    ]
    return sharded, args


# revision 17
# speedup vs baseline: 2.2294x; 1.3789x over previous
"""Trainium2 Bass kernel for a 16-head causal MultiHeadAttention block.

Problem (hardcoded): B=4, S=2048, D=1024, H=16, DK=64, fp32 I/O.
    out = softmax(mask(Q' K'^T / sqrt(DK))) V' @ Wo.T + bo
with Q' = Q@Wq.T+bq etc.

Sharding: 8 cores = (batch b = core//2, head-half = core%2).  Each core
computes its batch's q/k/v projections for its 8 heads, causal attention,
and a partial output projection over its 512 attn dims.  The host sums the
two partial outputs per batch (the Wo contraction distributes over heads).

Per-core kernel layout (v4, tuned against HW microbenches: all matmul
dtypes stream ~1 col/cycle at 2.4 GHz; fp8 DoubleRow contracts 256
rows/instr at 1 col/cycle; DR LDWEIGHTS loads 2x columns so DR only pays
off on wide streams; fp8/bf16 non-DR LDWEIGHTS get FWL 4x):
  - bf16 data path everywhere except: q/k projections for sequence tiles
    st >= 1 run fp8e4m3 DoubleRow (256-row contraction - half the
    instructions).  Host emulation: the fp8 score noise only hurts
    low-key-count (early) rows, so st=0 stays bf16; max rel err 4.3e-3
    vs the 2e-2 budget.  fp8 V or output projections fail the budget
    (their noise hits the output directly) so those stay bf16.
  - Host pre-transposes activations (X^T [D,S]) / weights and pre-scales
    Wq, Wk by 2^8 (fp8 resolution); exp scale 2^-19 folds that and
    1/sqrt(DK).
  - q/k land transposed [head_dim, seq] in bf16; scores are computed
    transposed (scoresT[k, q] = kT.T @ qT) so exp output feeds PV
    directly as lhsT.
  - Score matmuls for the head-pair's two heads (partitions 0:64 / 64:128)
    are issued adjacently: they map to different PE row-groups and run
    concurrently (~2x).  Each (jp, s_) score tile [128, 2, 512] holds both
    heads; one exp instruction (N=1024) covers both, bias -1.5 (softmax
    shift-invariance; keeps exp small, scores bounded |s| < 6.5).
  - A slice of the full-tile exps runs on the DVE instead (Schraudolph
    int-bit-trick exp, ~3% rel err, inside the bf16-path budget) to
    offload the Activation engine.
  - PV runs bf16 with per-j v tiles [128, HPC, DK+1]; the ones column
    accumulates the softmax denominator for free.
  - Diagonal 128x128 blocks: exp for both heads in one instruction,
    triangular zero mask via gpsimd affine_select.
  - Causal structure at tile granularity: upper-triangle k-tiles skipped.
  - Normalization fused: one scalar_tensor_tensor per (hp, sub) with a
    broadcast reciprocal; bf16 PE transposes (1 cyc/row) into a single
    psum tile; one attnT copy per (qt, hp).
"""

import math
import contextlib

import numpy as np
import ml_dtypes
import concourse.bacc as bacc
import concourse.tile as tile
from concourse import mybir
from concourse import bass_utils
from concourse.masks import make_identity
from concourse.bass import broadcast_tensor_aps

B, S, D, H = 4, 2048, 1024, 16
DK = D // H            # 64
NCORES = 8
HPC = H // 2           # 8 heads per core
DHC = HPC * DK         # 512 attn dims per core
KD = D // 128          # 8 contraction chunks (bf16 path)
KD2 = D // 256         # 4 DoubleRow contraction chunks (fp8 path)
NPAIR = HPC // 2       # 4 head pairs per core
EXPBIAS = -1.5
SCALE_QK = float(2.0 ** -19)   # 1/(2^8 * 2^8 * sqrt(DK))
_A_SCH = 12102203.16158        # 2^23/ln(2)
SCHRA_A = _A_SCH * SCALE_QK
SCHRA_B = float(127 * 2 ** 23 - 366393 + EXPBIAS * _A_SCH)
QK_FP8_ST0 = 1                 # first seq tile using the fp8 qk path
SCHRA_MOD, SCHRA_LIM = 14, 5   # DVE-exp share of full-tile exps
INTERLEAVE_PROJ = True         # issue st>=1 projections inside attention

F32 = mybir.dt.float32
F32R = mybir.dt.float32r
BF16 = mybir.dt.bfloat16
FP8 = mybir.dt.float8e4
I32 = mybir.dt.int32
NPF8 = ml_dtypes.float8_e4m3
NPBF16 = ml_dtypes.bfloat16
DR = mybir.MatmulPerfMode.DoubleRow


def build_nc(seq=S, causal=True, repeat=1):
    nc = bacc.Bacc("TRN2", target_bir_lowering=False, debug=False)

    NKC = seq // 128   # 128-wide k/seq tiles
    NQT = seq // 512   # 512-wide q tiles
    NJP = NKC // 2     # j-block pairs

    xq = nc.dram_tensor("xqT", [D, seq], BF16, kind="ExternalInput").ap()
    xk = nc.dram_tensor("xkT", [D, seq], BF16, kind="ExternalInput").ap()
    xv = nc.dram_tensor("xvT", [D, seq], BF16, kind="ExternalInput").ap()
    xq8 = nc.dram_tensor("xqT8", [D, seq], FP8, kind="ExternalInput").ap()
    xk8 = nc.dram_tensor("xkT8", [D, seq], FP8, kind="ExternalInput").ap()
    wq = nc.dram_tensor("wqT", [D, DHC], BF16, kind="ExternalInput").ap()
    wk = nc.dram_tensor("wkT", [D, DHC], BF16, kind="ExternalInput").ap()
    wq8 = nc.dram_tensor("wqT8", [D, DHC], FP8, kind="ExternalInput").ap()
    wk8 = nc.dram_tensor("wkT8", [D, DHC], FP8, kind="ExternalInput").ap()
    wv = nc.dram_tensor("wvT", [D, DHC], BF16, kind="ExternalInput").ap()
    wo = nc.dram_tensor("woT", [DHC, D], BF16, kind="ExternalInput").ap()
    out = nc.dram_tensor("out", [seq, D], BF16, kind="ExternalOutput").ap()

    EXP = mybir.ActivationFunctionType.Exp

    with tile.TileContext(nc) as tc, contextlib.ExitStack() as ctx:
        ep = ctx.enter_context

        consts = ep(tc.tile_pool(name="consts", bufs=1))
        wpool = ep(tc.tile_pool(name="wpool", bufs=2))
        w8pool = ep(tc.tile_pool(name="w8pool", bufs=2))
        wopool = ep(tc.tile_pool(name="wopool", bufs=1))
        xpool = ep(tc.tile_pool(name="xpool", bufs=4))
        qtp = ep(tc.tile_pool(name="qtp", bufs=NPAIR))
        ktp = ep(tc.tile_pool(name="ktp", bufs=NPAIR))
        vbp = ep(tc.tile_pool(name="vbp", bufs=NKC))
        atp = ep(tc.tile_pool(name="atp", bufs=2 * NPAIR))
        pt2p = ep(tc.tile_pool(name="pt2p", bufs=6))
        ptdp = ep(tc.tile_pool(name="ptdp", bufs=4))
        stgp = ep(tc.tile_pool(name="stgp", bufs=4))
        outp = ep(tc.tile_pool(name="outp", bufs=4))
        rcp = ep(tc.tile_pool(name="rcp", bufs=4))
        sstp = ep(tc.tile_pool(name="sstp", bufs=3))
        psA = ep(tc.tile_pool(name="psA", bufs=2, space="PSUM"))
        psB = ep(tc.tile_pool(name="psB", bufs=2, space="PSUM"))
        psD = ep(tc.tile_pool(name="psD", bufs=1, space="PSUM"))

        ident = consts.tile([128, 128], BF16)
        make_identity(nc, ident)
        ebias = consts.tile([128, 1], F32, tag="ebias", name="ebias")
        nc.gpsimd.memset(ebias, EXPBIAS)

        for rep_i in range(repeat):
            def load_w16(wdram, dma_eng):
                wsb = wpool.tile([128, KD, DHC], BF16, tag="w", name="wsb")
                dma_eng.dma_start(
                    out=wsb, in_=wdram.rearrange("(c p) m -> p c m", p=128))
                return wsb

            def load_w8(wdram, dma_eng):
                wsb = w8pool.tile([128, KD2, 2, DHC], FP8, tag="w8",
                                  name="wsb8")
                dma_eng.dma_start(
                    out=wsb,
                    in_=wdram.rearrange("(c two p) m -> p c two m",
                                        p=128, two=2))
                return wsb

            wq_sb = load_w16(wq, nc.scalar)
            wk_sb = load_w16(wk, nc.gpsimd)
            wq8_sb = load_w8(wq8, nc.scalar)
            wk8_sb = load_w8(wk8, nc.gpsimd)
            wo_sb = wopool.tile([128, DHC // 128, D], BF16)
            nc.sync.dma_start(out=wo_sb, in_=wo.rearrange("(c p) n -> p c n", p=128))
            # (out DMAs below keep nc.sync mostly to themselves)

            qT = [qtp.tile([128, seq], BF16, tag="qT", name=f"qT{i}") for i in range(NPAIR)]
            kT = [ktp.tile([128, seq], BF16, tag="kT", name=f"kT{i}") for i in range(NPAIR)]

            # ---- q / k projections: bf16 for st < QK_FP8_ST0, fp8
            # DoubleRow beyond.  psum[dpair, s] = sum_i W[i, dpair] X[i, s]
            def qk_proj(dst, xdram, x8dram, wsb, w8sb, dma_eng):
                for st in range(seq // 512):
                    f8path = st >= QK_FP8_ST0
                    if f8path:
                        xt = xpool.tile([128, KD2, 2, 512], FP8, tag="x",
                                        name="xt8")
                        dma_eng.dma_start(
                            out=xt,
                            in_=x8dram[:, st * 512:(st + 1) * 512].rearrange(
                                "(c two p) s -> p c two s", p=128, two=2))
                    else:
                        xt = xpool.tile([128, KD, 512], BF16, tag="x",
                                        name="xt")
                        dma_eng.dma_start(
                            out=xt,
                            in_=xdram[:, st * 512:(st + 1) * 512].rearrange(
                                "(c p) s -> p c s", p=128))
                    for pair in range(NPAIR):
                        ps = psA.tile([128, 512], F32, tag="psA", name="psp")
                        if f8path:
                            for kc in range(KD2):
                                nc.tensor.matmul(
                                    ps,
                                    lhsT=w8sb[:, kc, :,
                                              pair * 128:(pair + 1) * 128],
                                    rhs=xt[:, kc, :, :],
                                    start=(kc == 0), stop=(kc == KD2 - 1),
                                    perf_mode=DR,
                                )
                        else:
                            for kc in range(KD):
                                nc.tensor.matmul(
                                    ps,
                                    lhsT=wsb[:, kc,
                                             pair * 128:(pair + 1) * 128],
                                    rhs=xt[:, kc, :],
                                    start=(kc == 0), stop=(kc == KD - 1),
                                )
                        nc.vector.tensor_copy(
                            out=dst[pair][:, st * 512:(st + 1) * 512], in_=ps)

            def qk_units(dst, xdram, x8dram, wsb, w8sb, dma_eng, st):
                f8path = st >= QK_FP8_ST0
                if f8path:
                    xt = xpool.tile([128, KD2, 2, 512], FP8, tag="x",
                                    name="xt8")
                    dma_eng.dma_start(
                        out=xt,
                        in_=x8dram[:, st * 512:(st + 1) * 512].rearrange(
                            "(c two p) s -> p c two s", p=128, two=2))
                else:
                    xt = xpool.tile([128, KD, 512], BF16, tag="x", name="xt")
                    dma_eng.dma_start(
                        out=xt,
                        in_=xdram[:, st * 512:(st + 1) * 512].rearrange(
                            "(c p) s -> p c s", p=128))

                def unit(pair):
                    ps = psA.tile([128, 512], F32, tag="psA", name="psp")
                    if f8path:
                        for kc in range(KD2):
                            nc.tensor.matmul(
                                ps,
                                lhsT=w8sb[:, kc, :,
                                          pair * 128:(pair + 1) * 128],
                                rhs=xt[:, kc, :, :],
                                start=(kc == 0), stop=(kc == KD2 - 1),
                                perf_mode=DR,
                            )
                    else:
                        for kc in range(KD):
                            nc.tensor.matmul(
                                ps,
                                lhsT=wsb[:, kc, pair * 128:(pair + 1) * 128],
                                rhs=xt[:, kc, :],
                                start=(kc == 0), stop=(kc == KD - 1),
                            )
                    nc.vector.tensor_copy(
                        out=dst[pair][:, st * 512:(st + 1) * 512], in_=ps)
                return [lambda pair=pair: unit(pair) for pair in range(NPAIR)]

            # ---- v projection (natural layout, bf16) + ones column
            wv_sb = load_w16(wv, nc.gpsimd)
            vb = [None] * NKC

            def v_units(st):
                xt = xpool.tile([128, KD, 512], BF16, tag="x", name="xt")
                nc.gpsimd.dma_start(
                    out=xt,
                    in_=xv[:, st * 512:(st + 1) * 512].rearrange(
                        "(c p) s -> p c s", p=128))

                def unit(sq):
                    j = st * 4 + sq
                    ps = psA.tile([128, 512], F32, tag="psA", name="psp")
                    for kc in range(KD):
                        nc.tensor.matmul(
                            ps,
                            lhsT=xt[:, kc, sq * 128:(sq + 1) * 128],
                            rhs=wv_sb[:, kc, :],
                            start=(kc == 0), stop=(kc == KD - 1),
                        )
                    psh = ps.rearrange("p (h d) -> p h d", h=HPC)
                    vb[j] = vbp.tile([128, HPC, DK + 1], BF16, tag="vb",
                                     name="vbt")
                    nc.gpsimd.memset(vb[j][:, :, DK:DK + 1], 1.0)
                    nc.vector.tensor_copy(out=vb[j][:, :, 0:DK], in_=psh)
                return [lambda sq=sq: unit(sq) for sq in range(4)]

            # st=0 projections run eagerly (attention(qt=0) needs them);
            # later sequence tiles are interleaved into the attention loop
            # below so projection matmuls fill the PE while the Activation
            # engine works through the exp stream.
            pending = []
            for u in qk_units(qT, xq, xq8, wq_sb, wq8_sb, nc.scalar, 0):
                u()
            for u in qk_units(kT, xk, xk8, wk_sb, wk8_sb, nc.gpsimd, 0):
                u()
            for u in v_units(0):
                u()
            for st in range(1, seq // 512):
                pending.extend(qk_units(qT, xq, xq8, wq_sb, wq8_sb,
                                        nc.scalar, st))
                pending.extend(qk_units(kT, xk, xk8, wk_sb, wk8_sb,
                                        nc.gpsimd, st))
                pending.extend(v_units(st))
            # 36 pending units over 16 (qt, hp) iterations; the st=qt+1
            # slice is issued during attention(qt): 12 units per qt = 3/hp
            if not INTERLEAVE_PROJ:
                for u in pending:
                    u()
                pending = []

            if not causal:
                for u in pending:
                    u()
            # ---- attention + output projection, one 512-wide q tile at a time
            for qt in range(NQT):
                njp = 2 * qt if causal else NJP
                attnT = [atp.tile([128, 512], BF16, tag="attnT", name=f"attnT{i}") for i in range(NPAIR)]
                for hp in range(NPAIR):
                    if causal:
                        base = qt * 12
                        hp_units = list(
                            pending[base + hp * 3:base + hp * 3 + 3])
                    else:
                        hp_units = []

                    def drip(n=1):
                        for _ in range(n):
                            if hp_units:
                                hp_units.pop(0)()
                    stg = stgp.tile([128, 4, 128], BF16, tag="stg", name="stg")
                    pvs = [psB.tile([128, 4, DK + 1], F32, tag="psB", name="pvt")
                           for _ in range(2)]
                    first = True
                    # full 128-key j tiles: both heads' scores in one
                    # 2-bank psum tile -> one exp -> bf16 FWL PV matmuls
                    for jp in range(njp):
                        for s_ in range(2):
                            j = 2 * jp + s_
                            sc = psA.tile([128, 2, 512], F32, tag="psA",
                                          name="sct")
                            for sub in range(2):
                                row0 = sub * 64
                                nc.tensor.matmul(
                                    sc[:, sub, :],
                                    lhsT=kT[hp][row0:row0 + 64,
                                                j * 128:(j + 1) * 128],
                                    rhs=qT[hp][row0:row0 + 64,
                                               qt * 512:(qt + 1) * 512],
                                    start=True,
                                    stop=True,
                                )
                            pt2 = pt2p.tile([128, 2, 512], BF16, tag="pt2",
                                            name="pt2")
                            if (j * 3 + hp) % SCHRA_MOD < SCHRA_LIM:
                                # Schraudolph exp on the DVE: int bit-trick
                                #   i32 = A*u + B;  f32-bits(i32) ~ e^u
                                ss = sstp.tile([128, 2, 512], I32,
                                               tag="ss", name="ss")
                                nc.vector.tensor_scalar(
                                    out=ss, in0=sc,
                                    scalar1=SCHRA_A, scalar2=SCHRA_B,
                                    op0=mybir.AluOpType.mult,
                                    op1=mybir.AluOpType.add)
                                nc.vector.tensor_copy(
                                    out=pt2, in_=ss.bitcast(F32))
                            else:
                                nc.scalar.activation(pt2, sc, EXP, bias=ebias,
                                                     scale=SCALE_QK)
                            for sub in range(2):
                                h = hp * 2 + sub
                                for c in range(4):
                                    nc.tensor.matmul(
                                        pvs[sub][:, c, :],
                                        lhsT=pt2[:, sub,
                                                 c * 128:(c + 1) * 128],
                                        rhs=vb[j][:, h, :],
                                        start=first and c == 0,
                                        stop=(not causal and jp == njp - 1
                                              and s_ == 1 and c == 3),
                                    )
                            first = False
                            if s_ == 1 and jp < 3:
                                drip()
                    # diagonal blocks: both heads per exp, triangular
                    # mask on the 128x128 diagonal block
                    if causal:
                        drip(3)
                        for d in range(4):
                            j = 4 * qt + d
                            qoff = d * 128
                            w = 512 - qoff
                            sc2 = psD.tile([128, 2, 512], F32, tag="psD",
                                           name="scd")
                            for sub in range(2):
                                row0 = sub * 64
                                nc.tensor.matmul(
                                    sc2[:, sub, qoff:512],
                                    lhsT=kT[hp][row0:row0 + 64,
                                                j * 128:(j + 1) * 128],
                                    rhs=qT[hp][row0:row0 + 64,
                                               qt * 512 + qoff:(qt + 1) * 512],
                                    start=True,
                                    stop=True,
                                )
                            ptd = ptdp.tile([128, 2, 512], BF16, tag="ptd",
                                            name="ptd")
                            nc.scalar.activation(
                                ptd[:, :, 0:w], sc2[:, :, qoff:512], EXP,
                                bias=ebias, scale=SCALE_QK)
                            # diagonal 128x128 block: zero p where k > q
                            for sub in range(2):
                                nc.gpsimd.affine_select(
                                    out=ptd[:, sub, 0:128],
                                    in_=ptd[:, sub, 0:128],
                                    compare_op=mybir.AluOpType.is_ge,
                                    fill=0.0,
                                    base=0,
                                    channel_multiplier=-1,
                                    pattern=[[1, 128]],
                                )
                            for sub in range(2):
                                h = hp * 2 + sub
                                for c in range(d, 4):
                                    nc.tensor.matmul(
                                        pvs[sub][:, c, :],
                                        lhsT=ptd[:, sub, c * 128 - qoff:
                                                 c * 128 - qoff + 128],
                                        rhs=vb[j][:, h, :],
                                        start=first and d == 0 and c == d,
                                        stop=(d == 3 and c == 3),
                                    )
                            first = False
                    for sub in range(2):
                        row0 = sub * 64
                        rc4 = rcp.tile([128, 4, 1], F32, tag="rc", name="rc")
                        nc.vector.reciprocal(rc4, pvs[sub][:, :, DK:DK + 1])
                        pv_in = pvs[sub][:, :, 0:DK]
                        rc_b, pv_b = broadcast_tensor_aps(
                            rc4[:, :, 0:1], pv_in)
                        nc.vector.scalar_tensor_tensor(
                            out=stg[:, :, row0:row0 + 64],
                            in0=pv_b, scalar=1.0, in1=rc_b,
                            op0=mybir.AluOpType.mult,
                            op1=mybir.AluOpType.mult)
                    tpv = psB.tile([128, 4, 128], BF16, tag="psB", name="tp")
                    for c in range(4):
                        nc.tensor.transpose(tpv[:, c, :], stg[:, c, :], ident)
                    nc.vector.tensor_copy(
                        out=attnT[hp].rearrange("p (c n) -> p c n", c=4),
                        in_=tpv)

                for t in range(4):
                    row = (qt * 4 + t) * 128
                    ps2 = psD.tile([128, 2, 512], F32, tag="psD", name="pso")
                    for half in range(2):
                        for dc in range(NPAIR):
                            nc.tensor.matmul(
                                ps2[:, half, :],
                                lhsT=attnT[dc][:, t * 128:(t + 1) * 128],
                                rhs=wo_sb[:, dc, half * 512:(half + 1) * 512],
                                start=(dc == 0),
                                stop=(dc == NPAIR - 1),
                            )
                    og = outp.tile([128, D], BF16, tag="out")
                    ogv = og.rearrange("p (h n) -> p h n", h=2)
                    if qt == NQT - 1 and t % 2 == 1:
                        nc.scalar.activation(
                            ogv, ps2, mybir.ActivationFunctionType.Copy)
                    else:
                        nc.vector.tensor_copy(out=ogv, in_=ps2)
                    nc.sync.dma_start(out=out[row:row + 128, :], in_=og)

    nc.compile()
    return nc


_NC_CACHE = {}


def _get_nc(seq, causal, repeat=1):
    key = (seq, causal, repeat)
    if key not in _NC_CACHE:
        _NC_CACHE[key] = build_nc(seq, causal, repeat)
    return _NC_CACHE[key]


def shard_inputs(Q, K, V, Wq, Wk, Wv, Wo, seq=S):
    xT = {}
    for b in range(B):
        xT[b] = (
            np.asarray(Q[b][:seq].T, dtype=NPBF16, order="C"),
            np.asarray(K[b][:seq].T, dtype=NPBF16, order="C"),
            np.asarray(V[b][:seq].T, dtype=NPBF16, order="C"),
            np.asarray(Q[b][:seq].T, dtype=NPF8, order="C"),
            np.asarray(K[b][:seq].T, dtype=NPF8, order="C"),
        )
    wT = {}
    for hh in range(2):
        ds0 = hh * DHC
        wq_s = (Wq[ds0:ds0 + DHC] * 256.0).T
        wk_s = (Wk[ds0:ds0 + DHC] * 256.0).T
        wT[hh] = (
            np.asarray(wq_s, dtype=NPBF16, order="C"),
            np.asarray(wk_s, dtype=NPBF16, order="C"),
            np.asarray(wq_s, dtype=NPF8, order="C"),
            np.asarray(wk_s, dtype=NPF8, order="C"),
            np.asarray(Wv[ds0:ds0 + DHC].T, dtype=NPBF16, order="C"),
            np.asarray(Wo[:, ds0:ds0 + DHC].T, dtype=NPBF16, order="C"),
        )
    in_maps = []
    for c in range(NCORES):
        b, hh = c // 2, c % 2
        in_maps.append({
            "xqT": xT[b][0], "xkT": xT[b][1], "xvT": xT[b][2],
            "xqT8": xT[b][3], "xkT8": xT[b][4],
            "wqT": wT[hh][0], "wkT": wT[hh][1],
            "wqT8": wT[hh][2], "wkT8": wT[hh][3],
            "wvT": wT[hh][4], "woT": wT[hh][5],
        })
    return in_maps


def _numpy_ref(Q, K, V, mask, Wq, bq, Wk, bk, Wv, bv, Wo, bo):
    """Safety-net host fallback for input patterns the device kernel
    doesn't handle (non-causal non-empty masks, nonzero q/k biases)."""
    b = Q.shape[0]
    q = (Q @ Wq.T + bq).reshape(b, -1, H, DK).transpose(0, 2, 1, 3)
    k = (K @ Wk.T + bk).reshape(b, -1, H, DK).transpose(0, 2, 1, 3)
    v = (V @ Wv.T + bv).reshape(b, -1, H, DK).transpose(0, 2, 1, 3)
    scores = np.einsum("bhqd,bhkd->bhqk", q, k) / math.sqrt(DK)
    scores = np.where(mask, np.float32(-1e9), scores)
    scores -= scores.max(axis=-1, keepdims=True)
    p = np.exp(scores)
    p /= p.sum(axis=-1, keepdims=True)
    o = np.einsum("bhqk,bhkd->bhqd", p, v)
    o = o.transpose(0, 2, 1, 3).reshape(b, -1, H * DK)
    return (o @ Wo.T + bo).astype(np.float32)


def _run(inputs, trace=False):
    Q = np.asarray(inputs["Q"], np.float32)
    K = np.asarray(inputs["K"], np.float32)
    V = np.asarray(inputs["V"], np.float32)
    mask = np.asarray(inputs["mask"], bool)
    Wq = np.asarray(inputs["Wq"], np.float32)
    bq = np.asarray(inputs["bq"], np.float32)
    Wk = np.asarray(inputs["Wk"], np.float32)
    bk = np.asarray(inputs["bk"], np.float32)
    Wv = np.asarray(inputs["Wv"], np.float32)
    bv = np.asarray(inputs["bv"], np.float32)
    Wo = np.asarray(inputs["Wo"], np.float32)
    bo = np.asarray(inputs["bo"], np.float32)

    seq = Q.shape[1]
    m2 = mask[:, 0]
    triu = np.triu(np.ones((seq, seq), bool), 1)
    if all(np.array_equal(m2[i], triu) for i in range(m2.shape[0])):
        causal = True
    elif not mask.any():
        causal = False
    else:
        return _numpy_ref(Q, K, V, mask, Wq, bq, Wk, bk, Wv, bv, Wo, bo), None
    if bq.any() or bk.any():
        return _numpy_ref(Q, K, V, mask, Wq, bq, Wk, bk, Wv, bv, Wo, bo), None

    nc = _get_nc(seq, causal)
    in_maps = shard_inputs(Q, K, V, Wq, Wk, Wv, Wo, seq)
    res = bass_utils.run_bass_kernel_spmd(
        nc, in_maps, core_ids=list(range(NCORES)), trace=trace
    )
    outs = [np.asarray(r["out"], np.float32) for r in res.results]
    out = np.empty((B, seq, D), np.float32)
    for b in range(B):
        out[b] = outs[2 * b] + outs[2 * b + 1]
    # v-bias distributes through softmax (weights sum to 1); o-bias is direct
    out += bo + bv @ Wo.T
    return out, res


def kernel(**inputs):
    out, _ = _run(inputs)
    return out


def make_timed_runner(nc, in_maps):
    """Build a jitted shard_map callable over 8 cores with device-resident,
    non-donated inputs, for steady-state kernel timing (no NTFF hook is
    available under this axon client, so wall-clock the sharded executable)."""
    import jax
    from jax.experimental.shard_map import shard_map
    from jax.sharding import Mesh, NamedSharding, PartitionSpec
    from concourse import bass2jax
    from concourse import mybir as mb

    bass2jax.install_neuronx_cc_hook()

    partition_name = (
        nc.partition_id_tensor.name if nc.partition_id_tensor else None
    )
    in_names, out_names, out_avals, zero_outs = [], [], [], []
    for alloc in nc.m.functions[0].allocations:
        if not isinstance(alloc, mb.MemoryLocationSet):
            continue
        name = alloc.memorylocations[0].name
        if alloc.kind == "ExternalInput":
            if name != partition_name:
                in_names.append(name)
        elif alloc.kind == "ExternalOutput":
            out_names.append(name)
            out_avals.append(
                jax.core.ShapedArray(tuple(alloc.tensor_shape), mb.dt.np(alloc.dtype))
            )
            zero_outs.append(
                np.zeros(tuple(alloc.tensor_shape), mb.dt.np(alloc.dtype))
            )
    n_params = len(in_names)
    all_names = in_names + out_names
    if partition_name is not None:
        all_names = all_names + [partition_name]

    def _body(*args):
        operands = list(args)
        if partition_name is not None:
            operands.append(bass2jax.partition_id_tensor())
        outs = bass2jax._bass_exec_p.bind(
            *operands,
            out_avals=tuple(out_avals),
            in_names=tuple(all_names),
            out_names=tuple(out_names),
            lowering_input_output_aliases=(),
            sim_require_finite=True,
            sim_require_nnan=True,
            nc=nc,
        )
        return tuple(outs)

    n = len(in_maps)
    devices = jax.devices()[:n]
    mesh = Mesh(np.asarray(devices), ("core",))
    spec = PartitionSpec("core")
    sharded = jax.jit(
        shard_map(
            _body,
            mesh=mesh,
            in_specs=(spec,) * (n_params + len(out_names)),
            out_specs=(spec,) * len(out_names),
            check_rep=False,
        ),
        keep_unused=True,
    )
    sh = NamedSharding(mesh, spec)
    args = [
        jax.device_put(
            np.concatenate([np.asarray(m[nm]) for m in in_maps], axis=0), sh
        )
        for nm in in_names
    ] + [
        jax.device_put(
            np.zeros((n * z.shape[0], *z.shape[1:]), z.dtype), sh
        )
        for z in zero_outs
    ]
    return sharded, args
